# revision 1
# baseline (speedup 1.0000x reference)
"""Multi-head causal attention (B=2, T=2048, D=1024, H=16, dk=dv=64) on 8 NeuronCores.

Sharding: data parallel over batch (2) x tensor parallel over heads (4 groups of 4).
Core c handles batch c//4, heads [4*(c%4), 4*(c%4)+4). Each core computes the
partial output sum over its 4 heads; host adds the 4 partials per batch.

Per-core pipeline (everything transposed so no activation transposes are needed):
  QT/KT [256, T] = W.T @ xT        (fp32r matmuls, PSUM k-accumulation)
  VT    [256, T] -> PE-transpose -> V_aug [T, 65] per head (65th col = ones)
  per head, per tq-slice (512), per tk-tile (128), causal-skipped:
    ST block [tk 128, tq 512] = KT_h^T-slice.T @ QT_h      (K=64)
    diag blocks: += additive -1e30 mask (DVE on PSUM)
    ET = exp(0.125 * ST)  (ACT, PSUM->SBUF fp32r, batched over 2 blocks)
    OT_aug [65, 512] += V_aug_h.T-slice @ ET   (fused rowsum via ones col)
  normalization: rowsums -> DRAM bounce broadcast -> reciprocal -> OT scale
  out [T, 1024] = OT.T @ Wo  (partial over this core's 4 heads)
"""
import sys

sys.path.insert(0, "/opt/trn_rl_repo")

import functools
import os
import ml_dtypes
import numpy as np

import concourse.bass as bass
import concourse.tile as tile
from concourse import mybir
from concourse.bass_utils import run_bass_kernel_spmd

B, T, D = 2, 2048, 1024
H, DK = 16, 64            # total heads
HG = 4                    # heads per core
G = HG * DK               # 256: per-core column group width
NKT = D // 128            # 8 k-tiles of the model dim
NT = T // 128             # 16 tk tiles
NS = 4                    # tq slices
TQ = T // NS              # 512
NEG = -1e30
F32 = mybir.dt.float32
F32R = mybir.dt.float32r
BF16 = mybir.dt.bfloat16
IN_DT = BF16  # dtype for x / Wq / Wk / Wv (projection operands)


def split_multi_waits(nc, max_waits=1):
    """This walrus build has tiny per-instruction sync-wait slot limits (1 for
    fp32r matmul LW, ~2 for CTRL). Move excess waits onto preceding same-engine
    NOPs - identical semantics since each engine executes serially."""
    for func in nc.m.functions:
        for bb in func.blocks:
            out = []
            for inst in list(bb.instructions):
                si = inst.sync_info
                waits = list(si.on_wait) if (si and si.on_wait) else []
                if len(waits) > max_waits:
                    extra, keep = waits[:-max_waits], waits[-max_waits:]
                    for j, w in enumerate(extra):
                        nop = mybir.InstNoOp(name=f"{inst.name}-ws{j}")
                        nop.engine = inst.engine
                        nop.sync_info = mybir.SyncInfo(on_wait=[w], on_update=[])
                        out.append(nop)
                    inst.sync_info = mybir.SyncInfo(
                        on_wait=keep, on_update=list(si.on_update or []))
                out.append(inst)
            bb.instructions = out


def _n_alive(s, mode):
    """Number of tk tiles needed for tq slice s."""
    return NT if mode != "causal" else (TQ // 128) * (s + 1)


@functools.lru_cache(maxsize=4)
def build_program(mode, _env=None):
    assert mode in ("causal", "dense", "masked")
    nc = bass.Bass()
    qT = nc.dram_tensor("qT", [D, T], IN_DT, kind="ExternalInput")
    kTt = nc.dram_tensor("kT", [D, T], IN_DT, kind="ExternalInput")
    vT = nc.dram_tensor("vT", [D, T], IN_DT, kind="ExternalInput")
    # weights pre-packed on host into SBUF layout: [128, NKT*G] with
    # partition p holding wq[kk*128+p, :] at cols [kk*G, (kk+1)*G)
    wq = nc.dram_tensor("wq", [128, NKT * G], IN_DT, kind="ExternalInput")
    wk = nc.dram_tensor("wk", [128, NKT * G], IN_DT, kind="ExternalInput")
    wv = nc.dram_tensor("wv", [128, NKT * G], IN_DT, kind="ExternalInput")
    wo = nc.dram_tensor("wo", [128, 2 * D], BF16, kind="ExternalInput")
    out = nc.dram_tensor("out", [T, D], F32, kind="ExternalOutput")
    rdram = nc.dram_tensor("rdram", [HG, T], F32)
    DBG = bool(int(os.environ.get("KDBG", "0")))
    dbg = {}
    if DBG:
        for nm, shape, dt_ in [("dqt0", [128, T], BF16), ("dkt0", [128, T], BF16),
                               ("dva0", [128, NT * (DK + 1)], BF16),
                               ("dotu0", [128, T], F32R),
                               ("drd", [HG, T], F32),
                               ("dmasks", [128, 4 * TQ], F32),
                               ("det", [128, 2 * TQ], BF16),
                               ("det1", [128, 2 * TQ], BF16),
                               ("dva1", [128, NT * (DK + 1)], BF16),
                               ("dsps", [128, 2 * TQ], F32),
                               ("dsps1", [128, 2 * TQ], F32)]:
            dbg[nm] = nc.dram_tensor(nm, shape, dt_, kind="ExternalOutput")
    maskd = None
    if mode == "masked":
        maskd = nc.dram_tensor("maskT", [T, T], BF16, kind="ExternalInput")

    with tile.TileContext(nc) as tc:
        with (
            tc.tile_pool(name="sing", bufs=1) as sing,
            tc.tile_pool(name="xbig", bufs=1) as xbig,
            tc.tile_pool(name="etp", bufs=10) as etp,
            tc.tile_pool(name="ost", bufs=4) as ostp,
        ):
            # ---------------- constants ----------------
            wq_s = sing.tile([128, NKT * G], IN_DT)
            wk_s = sing.tile([128, NKT * G], IN_DT)
            wv_s = sing.tile([128, NKT * G], IN_DT)
            # DMA issue order follows need-time: wv before vTr (V-matmuls
            # consume both first); wq/wk after vTr; wo last (outproj only).
            nc.sync.dma_start(out=wv_s[:], in_=wv[:])
            wo_s = sing.tile([128, 2 * D], BF16)
            ones_sb = sing.tile([128, NT], BF16)
            nc.vector.memset(ones_sb[:], 1.0)
            if mode == "causal":
                # multiplicative 0/1 masks (bf16), applied to ET post-exp
                masks = sing.tile([128, 4 * TQ], BF16)
                nc.gpsimd.memset(masks[:], 1.0)
                for i in range(4):
                    nc.gpsimd.affine_select(
                        out=masks[:, i * TQ:(i + 1) * TQ],
                        in_=masks[:, i * TQ:(i + 1) * TQ],
                        compare_op=mybir.AluOpType.is_ge,
                        fill=0.0, base=-(128 * i), channel_multiplier=-1,
                        pattern=[[1, TQ]])

            qt = [sing.tile([128, T], BF16, name=f"qt{p}") for p in range(2)]
            kt = [sing.tile([128, T], BF16, name=f"kt{p}") for p in range(2)]
            va = [sing.tile([128, NT * (DK + 1)], BF16, name=f"va{h}")
                  for h in range(HG)]
            # aug-last: ones col at 64 of each 65-wide group (rowsum row)
            for h in range(HG):
                nc.vector.tensor_copy(va[h][:, DK::DK + 1], ones_sb[:])

            # ------- fused projections + attention (single scheduling region)
            otrs_cm = tc.tile_pool(name="otrs", bufs=1)
            otrs = otrs_cm.__enter__()
            otu = [[otrs.tile([128, TQ], BF16, name=f"otu{p}_{s}")
                    for s in range(NS)] for p in range(2)]
            with nc.named_scope("attn"), \
                 tc.tile_pool(name="sps", bufs=2, space="PSUM") as sps, \
                 tc.tile_pool(name="ops", bufs=2, space="PSUM") as ops, \
                 tc.tile_pool(name="mtp", bufs=4) as mtp, \
                 tc.tile_pool(name="rrp", bufs=4) as rrp:
                def emit_proj(xr, w_s, dst, m, only_n=None):
                    for n in ([only_n] if only_n is not None else range(NS)):
                        ps = qkps.tile([128, TQ], F32,
                                       name=f"pj{xr[0].tensor.name}{m}{n}",
                                       tag="qk")
                        for kk in range(NKT):
                            nc.tensor.matmul(
                                ps[:],
                                w_s[:, kk * G + m * 128: kk * G + (m + 1) * 128],
                                xr[kk][:, n * TQ:(n + 1) * TQ],
                                start=(kk == 0), stop=(kk == NKT - 1))
                        nc.vector.tensor_copy(
                            dst[m][:, n * TQ:(n + 1) * TQ], ps[:])

                def emit_head(h, s_list=None, ot_pools=None):
                    p, half = h // 2, h % 2
                    po = half * DK
                    for s in (range(NS) if s_list is None else s_list):
                        na = _n_alive(s, mode)
                        pool_i = (ot_pools or [ops])
                        opool = pool_i[s % len(pool_i)]
                        ot_ps = opool.tile([DK + 1, TQ], F32, name=f"ot{h}_{s}",
                                           tag=f"ot{opool.name}")
                        for tp2 in range(na // 2):
                            s_ps = sps.tile([128, 2 * TQ], F32,
                                            name=f"s{h}_{s}_{tp2}", tag="s")
                            # diag blocks: cols f < 128*d are masked for every
                            # partition -> skip in S/exp/mask/O. t=0 is always
                            # full width, so PSUM accumulation start covers all.
                            c0s, ds = [], []
                            for j in range(2):
                                t = 2 * tp2 + j
                                if mode == "causal" and t >= (TQ // 128) * s:
                                    d = (128 * t - TQ * s) // 128
                                    ds.append(d); c0s.append(128 * d)
                                else:
                                    ds.append(None); c0s.append(0)
                            for j in range(2):
                                t = 2 * tp2 + j
                                c0 = c0s[j]
                                nc.tensor.matmul(
                                    s_ps[:, j * TQ + c0:(j + 1) * TQ],
                                    kt[p][po:po + DK, t * 128:(t + 1) * 128],
                                    qt[p][po:po + DK, s * TQ + c0:(s + 1) * TQ],
                                    start=True, stop=True)
                            et = etp.tile([128, 2 * TQ], BF16,
                                          name=f"et{h}_{s}_{tp2}", tag="et")
                            if DBG and h == 0 and s == 0 and tp2 == 0:
                                dsp = sing.tile([128, 2 * TQ], F32)
                                nc.vector.tensor_copy(dsp[:], s_ps[:])
                                nc.sync.dma_start(out=dbg["dsps"][:], in_=dsp[:])
                            if DBG and h == 1 and s == 0 and tp2 == 0:
                                dsp1 = sing.tile([128, 2 * TQ], F32)
                                nc.vector.tensor_copy(dsp1[:], s_ps[:])
                                nc.sync.dma_start(out=dbg["dsps1"][:], in_=dsp1[:])
                            # One exp instruction costs ~352 extra cycles;
                            # splitting to skip dead columns only pays off when
                            # the skip is > 128 cols. For small c0 exp the dead
                            # region too (harmless: the O-matmul never reads
                            # it), starting at min(c0s).
                            if max(c0s) <= 128:
                                cm = min(c0s)
                                nc.scalar.activation(
                                    et[:, cm:2 * TQ], s_ps[:, cm:2 * TQ],
                                    mybir.ActivationFunctionType.Exp,
                                    scale=1.0 / np.sqrt(DK))
                            else:
                                for j in range(2):
                                    c0 = c0s[j]
                                    nc.scalar.activation(
                                        et[:, j * TQ + c0:(j + 1) * TQ],
                                        s_ps[:, j * TQ + c0:(j + 1) * TQ],
                                        mybir.ActivationFunctionType.Exp,
                                        scale=1.0 / np.sqrt(DK))
                            for j in range(2):
                                t = 2 * tp2 + j
                                if ds[j] is not None:
                                    d, c0 = ds[j], c0s[j]
                                    nc.vector.tensor_mul(
                                        et[:, j * TQ + c0:j * TQ + c0 + 128],
                                        et[:, j * TQ + c0:j * TQ + c0 + 128],
                                        masks[:, d * TQ + c0:d * TQ + c0 + 128])
                                elif mode == "masked":
                                    mt = mtp.tile([128, TQ], BF16,
                                                  name=f"mt{h}{s}{t}", tag="mt")
                                    nc.sync.dma_start(
                                        out=mt,
                                        in_=maskd[t * 128:(t + 1) * 128,
                                                  s * TQ:(s + 1) * TQ])
                                    nc.vector.tensor_mul(
                                        et[:, j * TQ:(j + 1) * TQ],
                                        et[:, j * TQ:(j + 1) * TQ], mt[:])
                            if DBG and h == 0 and s == 0 and tp2 == 0:
                                nc.sync.dma_start(out=dbg["det"][:], in_=et[:])
                            if DBG and h == 1 and s == 0 and tp2 == 0:
                                nc.sync.dma_start(out=dbg["det1"][:], in_=et[:])
                            for j in range(2):
                                t = 2 * tp2 + j
                                c0 = c0s[j]
                                nc.tensor.matmul(
                                    ot_ps[:, c0:TQ],
                                    va[h][:, t * (DK + 1):(t + 1) * (DK + 1)],
                                    et[:, j * TQ + c0:(j + 1) * TQ],
                                    start=(t == 0), stop=(t == na - 1))
                        nc.vector.tensor_copy(
                            otu[p][s][po:po + DK, :], ot_ps[0:DK, :])
                        rr = rrp.tile([128, TQ], F32, name=f"rr{h}_{s}",
                                      tag="rr")
                        nc.vector.tensor_copy(rr[DK:DK + 1, :],
                                              ot_ps[DK:DK + 1, :])
                        nc.sync.dma_start(
                            out=rdram[h:h + 1, s * TQ:(s + 1) * TQ],
                            in_=rr[DK:DK + 1, :])
                        if half == 1:
                            # both heads of the pair done for this slice:
                            # broadcast both rowsums, one recip + one mul.
                            rb = rrp.tile([128, TQ], F32, name=f"rb{h}_{s}",
                                          tag="rb")
                            nc.gpsimd.dma_start(
                                out=rb[0:DK, :],
                                in_=rdram[h - 1:h, s * TQ:(s + 1) * TQ]
                                .to_broadcast((DK, TQ)))
                            nc.gpsimd.dma_start(
                                out=rb[DK:128, :],
                                in_=rdram[h:h + 1, s * TQ:(s + 1) * TQ]
                                .to_broadcast((DK, TQ)))
                            nc.vector.reciprocal(rb[:], rb[:])
                            nc.vector.tensor_mul(
                                otu[p][s][:], otu[p][s][:], rb[:])

                # V first (heads' O-matmuls read va; emission order defines
                # RAW deps), then pair-0 projections + head 0, etc.
                # per-k-tile input tiles; DMA issue order v,q,k matches the
                # PE stream order (V-matmuls head the stream as the scheduler
                # preserves emission order per engine).
                vTr = [xbig.tile([128, T], IN_DT, name=f"vTr{kk}",
                                 tag=f"xv{kk}") for kk in range(NKT)]
                for kk in range(NKT):
                    nc.sync.dma_start(out=vTr[kk],
                                      in_=vT[kk * 128:(kk + 1) * 128, :])
                nc.sync.dma_start(out=wq_s[:], in_=wq[:])
                nc.sync.dma_start(out=wk_s[:], in_=wk[:])
                qTr = [xbig.tile([128, T], IN_DT, name=f"qTr{kk}", tag=f"xq{kk}")
                       for kk in range(NKT)]
                for kk in range(NKT):
                    nc.sync.dma_start(out=qTr[kk],
                                      in_=qT[kk * 128:(kk + 1) * 128, :])
                kTr = [xbig.tile([128, T], IN_DT, name=f"kTr{kk}", tag=f"xk{kk}")
                       for kk in range(NKT)]
                for kk in range(NKT):
                    nc.sync.dma_start(out=kTr[kk],
                                      in_=kTt[kk * 128:(kk + 1) * 128, :])
                nc.sync.dma_start(out=wo_s[:], in_=wo[:])
                # V in natural layout: per tk-tile, k-inner on resident vTr.
                # Emitted before the heads (O-matmuls read va -> RAW deps need
                # write-before-read emission order) but DEMOTED in priority so
                # the scheduler treats it as PE gap-filler under the ACT-bound
                # attention instead of running it ahead of the S-matmuls.
                with nc.named_scope("vproj"), \
                     tc.high_priority(offset=1000000), \
                     tc.tile_pool(name="vps", bufs=2, space="PSUM") as vps:
                    for t in range(NT):
                        ps = vps.tile([128, G], F32, name=f"v{t}", tag="v")
                        for kk in range(NKT):
                            nc.tensor.matmul(
                                ps[:],
                                vTr[kk][:, t * 128:(t + 1) * 128],
                                wv_s[:, kk * G:(kk + 1) * G],
                                start=(kk == 0), stop=(kk == NKT - 1))
                        for h in range(HG):
                            nc.vector.tensor_copy(
                                va[h][:, t * (DK + 1): t * (DK + 1) + DK],
                                ps[:, h * DK:(h + 1) * DK])
                qkps_cm = tc.tile_pool(name="qkps", bufs=2, space="PSUM")
                qkps = qkps_cm.__enter__()
                # interleave pair-0 projections, head-0 slices, and pair-1
                # projections at slice granularity: head0 slice s needs only
                # the n<=s projection groups, so attention starts as soon as
                # the first groups land; pair-1 projection work fills PE slack
                # under the ACT-bound attention.
                if mode == "causal":
                    # slice s of head 0 only reads the n<=s projection groups
                    for s in range(NS):
                        with nc.named_scope("proj"):
                            emit_proj(qTr, wq_s, qt, 0, only_n=s)
                            emit_proj(kTr, wk_s, kt, 0, only_n=s)
                        emit_head(0, s_list=[s])
                else:
                    # dense/masked: every slice reads all of kt - emit all
                    # pair-0 groups first
                    with nc.named_scope("proj"):
                        emit_proj(qTr, wq_s, qt, 0)
                        emit_proj(kTr, wk_s, kt, 0)
                    emit_head(0)
                # pair-1 projections interleaved into head 1 (pair 0): its
                # longer attention slices hide the projection PE time.
                for s in range(NS):
                    emit_head(1, s_list=[s])
                    with nc.named_scope("proj2"):
                        emit_proj(qTr, wq_s, qt, 1, only_n=s)
                        emit_proj(kTr, wk_s, kt, 1, only_n=s)
                qkps_cm.__exit__(None, None, None)
                with tc.tile_pool(name="ops2", bufs=2, space="PSUM") as ops2:
                    emit_head(2, ot_pools=[ops, ops2])
                # head 3 interleaved with the output projection: outproj for
                # tq slice s needs only otu[*][s], which is final once head 3
                # (the last head) finishes slice s.
                with tc.tile_pool(name="fps", bufs=2, space="PSUM") as fps:
                    for s in range(NS):
                        emit_head(3, s_list=[s])
                        with nc.named_scope("outproj"):
                            for m in range(4 * s, 4 * s + 4):
                                o_sb = ostp.tile([128, D], F32,
                                                 name=f"os{m}", tag="os")
                                for n in range(2):
                                    o_ps = fps.tile([128, TQ], F32,
                                                    name=f"op{m}_{n}", tag="op")
                                    for p in range(2):
                                        nc.tensor.matmul(
                                            o_ps[:],
                                            otu[p][s][:, (m % 4) * 128:
                                                      (m % 4 + 1) * 128],
                                            wo_s[:, p * D + n * TQ:
                                                 p * D + (n + 1) * TQ],
                                            start=(p == 0), stop=(p == 1))
                                    nc.vector.tensor_copy(
                                        o_sb[:, n * TQ:(n + 1) * TQ],
                                        o_ps[:])
                                nc.sync.dma_start(
                                    out=out[m * 128:(m + 1) * 128, :],
                                    in_=o_sb[:])

            if DBG:
                nc.sync.dma_start(out=dbg["dmasks"][:], in_=masks[:])
                nc.sync.dma_start(out=dbg["dqt0"][:], in_=qt[0][:])
                nc.sync.dma_start(out=dbg["dkt0"][:], in_=kt[0][:])
                nc.sync.dma_start(out=dbg["dva0"][:], in_=va[0][:])
                nc.sync.dma_start(out=dbg["dva1"][:], in_=va[1][:])
                nc.sync.dma_start(out=dbg["dotu0"][:], in_=otu[0][:])
                nc.sync.dma_start(out=dbg["drd"][:], in_=rdram[:])

            otrs_cm.__exit__(None, None, None)

    split_multi_waits(nc)
    return nc


def _detect_mode(mask):
    if mask.all():
        return "dense"
    if np.array_equal(mask, np.tril(np.ones((T, T), dtype=bool))):
        return "causal"
    return "masked"


def kernel(q, k, v, mask, Wq, Wk, Wv, Wo, _trace=False, _trace_kwargs=None):
    q, k, v = np.asarray(q), np.asarray(k), np.asarray(v)
    Wq, Wk, Wv, Wo = (np.asarray(Wq), np.asarray(Wk),
                      np.asarray(Wv), np.asarray(Wo))
    mask = np.asarray(mask)
    mode = _detect_mode(mask)
    nc = build_program(mode)

    in_maps = []
    for c in range(8):
        b, g = c // 4, c % 4
        bf = ml_dtypes.bfloat16

        def packw(w):  # [D, G] -> [128, NKT*G] SBUF layout
            return np.ascontiguousarray(
                w.reshape(NKT, 128, G).transpose(1, 0, 2).reshape(128, NKT * G)
                .astype(bf))

        wo_sl = Wo[g * G:(g + 1) * G, :]
        im = {
            "qT": np.ascontiguousarray(q[b].T.astype(bf)),
            "kT": np.ascontiguousarray(k[b].T.astype(bf)),
            "vT": np.ascontiguousarray(v[b].T.astype(bf)),
            "wq": packw(Wq[:, g * G:(g + 1) * G]),
            "wk": packw(Wk[:, g * G:(g + 1) * G]),
            "wv": packw(Wv[:, g * G:(g + 1) * G]),
            "wo": np.ascontiguousarray(
                wo_sl.reshape(2, 128, D).transpose(1, 0, 2).reshape(128, 2 * D)
                .astype(bf)),
        }
        if mode == "masked":
            im["maskT"] = np.ascontiguousarray(
                mask.T.astype(ml_dtypes.bfloat16))
        in_maps.append(im)

    res = run_bass_kernel_spmd(nc, in_maps, list(range(8)), trace=_trace,
                               **(_trace_kwargs or {}))
    outs = [res.results[c]["out"] for c in range(8)]
    full = np.stack([outs[4 * b] + outs[4 * b + 1] + outs[4 * b + 2]
                     + outs[4 * b + 3] for b in range(B)])
    if _trace:
        return full.astype(np.float32), res
    return full.astype(np.float32)



# revision 2
# speedup vs baseline: 1.0927x; 1.0927x over previous
"""Multi-head causal attention (B=2, T=2048, D=1024, H=16, dk=dv=64) on 8 NeuronCores.

Sharding: data parallel over batch (2) x tensor parallel over heads (4 groups of 4).
Core c handles batch c//4, heads [4*(c%4), 4*(c%4)+4). Each core computes the
partial output sum over its 4 heads; host adds the 4 partials per batch.

Per-core pipeline (the O-matmul uses et as STATIONARY and V_aug [128 keys, 65]
as MOVING -> 65 moving-cols per (key-tile x query-tile) instead of 512):
  QT/KT [256, T] = W.T @ xT        (bf16 matmuls, PSUM k-accumulation)
  V_aug [T, 65/head] (65th col = ones) via direct [T-part, G] projection
  per head, per tq-slice (512), per tk-tile-pair (2x128), causal-skipped:
    ST block [tk 128, tq 512] = KT_h-slice @ QT_h      (K=64)
    ET = exp(0.125 * ST)  (ACT, PSUM->SBUF bf16, batched over 2 blocks)
    diag blocks: ET *= 0/1 mask (DVE)
  per q-tile (128): O_aug [128 q, 65] += et-slice.T @ V_aug_h  (M=65 matmuls,
    et stationary); O-block lags the S-block by one slice to hide exp latency
    normalize: stg = O[:, 0:64] * recip(O[:, 64]) (per-partition scalar)
  per pair, q-tile: PE-transpose stg [q, dv2] -> otu [dv2, q]
  out [T, 1024] = otu.T @ Wo  (partial over this core's 4 heads, bf16 out)
"""
import sys

sys.path.insert(0, "/opt/trn_rl_repo")

import functools
import os
import ml_dtypes
import numpy as np

import concourse.bass as bass
import concourse.tile as tile
from concourse import mybir
from concourse.bass_utils import run_bass_kernel_spmd
from concourse.masks import make_identity

B, T, D = 2, 2048, 1024
H, DK = 16, 64            # total heads
HG = 4                    # heads per core
G = HG * DK               # 256: per-core column group width
NKT = D // 128            # 8 k-tiles of the model dim
NT = T // 128             # 16 tk tiles
NS = 4                    # tq slices
TQ = T // NS              # 512
NQT = TQ // 128           # 4 q-tiles per slice
F32 = mybir.dt.float32
BF16 = mybir.dt.bfloat16
IN_DT = BF16  # dtype for x / Wq / Wk / Wv (projection operands)
N_WARMUP = int(os.environ.get("KWARM", "400"))      # cap on filler warmups
USE_DIV = bool(int(os.environ.get("KDIV", "0")))
PAIR_BUDGET = float(os.environ.get("KPB", "450"))   # filler ns per S-pair
PE_CY = 1.0 / 2.4                                   # ns per PE cycle (ramped)


def split_multi_waits(nc, max_waits=1):
    """This walrus build has tiny per-instruction sync-wait slot limits (1 for
    matmul LW, ~2 for CTRL). Move excess waits onto preceding same-engine
    NOPs - identical semantics since each engine executes serially."""
    for func in nc.m.functions:
        for bb in func.blocks:
            out = []
            for inst in list(bb.instructions):
                si = inst.sync_info
                waits = list(si.on_wait) if (si and si.on_wait) else []
                if len(waits) > max_waits:
                    extra, keep = waits[:-max_waits], waits[-max_waits:]
                    for j, w in enumerate(extra):
                        nop = mybir.InstNoOp(name=f"{inst.name}-ws{j}")
                        nop.engine = inst.engine
                        nop.sync_info = mybir.SyncInfo(on_wait=[w], on_update=[])
                        out.append(nop)
                    inst.sync_info = mybir.SyncInfo(
                        on_wait=keep, on_update=list(si.on_update or []))
                out.append(inst)
            bb.instructions = out


def _n_alive(s, mode):
    """Number of tk tiles needed for tq slice s."""
    return NT if mode != "causal" else (TQ // 128) * (s + 1)


def _n_alive_qt(s, qt, mode):
    """Number of tk tiles needed for q-tile qt of slice s (128-granular)."""
    return NT if mode != "causal" else (TQ // 128) * s + qt + 1


@functools.lru_cache(maxsize=4)
def build_program(mode, _env=None):
    assert mode in ("causal", "dense", "masked")
    nc = bass.Bass()
    qT = nc.dram_tensor("qT", [D, T], IN_DT, kind="ExternalInput")
    kTt = nc.dram_tensor("kT", [D, T], IN_DT, kind="ExternalInput")
    vT = nc.dram_tensor("vT", [D, T], IN_DT, kind="ExternalInput")
    # weights pre-packed on host into SBUF layout: [128, NKT*G] with
    # partition p holding wq[kk*128+p, :] at cols [kk*G, (kk+1)*G)
    wq = nc.dram_tensor("wq", [128, NKT * G], IN_DT, kind="ExternalInput")
    wk = nc.dram_tensor("wk", [128, NKT * G], IN_DT, kind="ExternalInput")
    wv = nc.dram_tensor("wv", [128, NKT * G], IN_DT, kind="ExternalInput")
    wo = nc.dram_tensor("wo", [128, 2 * D], BF16, kind="ExternalInput")
    out = nc.dram_tensor("out", [T, D], BF16, kind="ExternalOutput")
    DBG = bool(int(os.environ.get("KDBG", "0")))
    dbg = {}
    if DBG:
        for nm, shape, dt_ in [("dqt0", [128, T], BF16),
                               ("dkt0", [128, T], BF16),
                               ("dva0", [128, NT * (DK + 1)], BF16),
                               ("dstg00", [128, NQT * 128], BF16),
                               ("dotu00", [128, TQ], BF16),
                               ("det", [128, 2 * TQ], BF16)]:
            dbg[nm] = nc.dram_tensor(nm, shape, dt_, kind="ExternalOutput")
    maskd = None
    if mode == "masked":
        maskd = nc.dram_tensor("maskT", [T, T], BF16, kind="ExternalInput")

    with tile.TileContext(nc) as tc:
        with (
            tc.tile_pool(name="sing", bufs=1) as sing,
            tc.tile_pool(name="xbig", bufs=1) as xbig,
            tc.tile_pool(name="etp", bufs=18) as etp,
            tc.tile_pool(name="ost", bufs=4) as ostp,
            tc.tile_pool(name="rcpp", bufs=4) as rcpp,
        ):
            # ---------------- constants ----------------
            wq_s = sing.tile([128, NKT * G], IN_DT)
            wk_s = sing.tile([128, NKT * G], IN_DT)
            wv_s = sing.tile([128, NKT * G], IN_DT)
            # DMA issue order follows need-time: wv before vTr (V-matmuls
            # consume both first); wq/wk after vTr; wo last (outproj only).
            nc.sync.dma_start(out=wv_s[:], in_=wv[:])
            wo_s = sing.tile([128, 2 * D], BF16)
            ones_sb = sing.tile([128, NT], BF16)
            nc.vector.memset(ones_sb[:], 1.0)
            ident = sing.tile([128, 128], BF16)
            make_identity(nc, ident[:])
            warm = sing.tile([128, 128], BF16)
            nc.vector.memset(warm[:], 0.0)
            if mode == "causal":
                # multiplicative 0/1 masks (bf16), applied to ET post-exp
                masks = sing.tile([128, 4 * TQ], BF16)
                nc.gpsimd.memset(masks[:], 1.0)
                for i in range(4):
                    nc.gpsimd.affine_select(
                        out=masks[:, i * TQ:(i + 1) * TQ],
                        in_=masks[:, i * TQ:(i + 1) * TQ],
                        compare_op=mybir.AluOpType.is_ge,
                        fill=0.0, base=-(128 * i), channel_multiplier=-1,
                        pattern=[[1, TQ]])

            qt = [sing.tile([128, T], BF16, name=f"qt{p}") for p in range(2)]
            kt = [sing.tile([128, T], BF16, name=f"kt{p}") for p in range(2)]
            va = [sing.tile([128, NT * (DK + 1)], BF16, name=f"va{h}")
                  for h in range(HG)]
            # aug-last: ones col at 64 of each 65-wide group (rowsum row)
            for h in range(HG):
                nc.vector.tensor_copy(va[h][:, DK::DK + 1], ones_sb[:])
            # normalized per-(pair, slice) outputs [q, dv-pair], q-tile major
            stg = [[sing.tile([128, NQT * 128], BF16, name=f"stg{p}_{s}")
                    for s in range(NS)] for p in range(2)]

            # ------- fused projections + attention (single scheduling region)
            otrs_cm = tc.tile_pool(name="otrs", bufs=1)
            otrs = otrs_cm.__enter__()
            otu = [[otrs.tile([128, TQ], BF16, name=f"otu{p}_{s}")
                    for s in range(NS)] for p in range(2)]
            with nc.named_scope("attn"), \
                 tc.tile_pool(name="sps", bufs=2, space="PSUM") as sps, \
                 tc.tile_pool(name="mtp", bufs=4) as mtp:
                etl = {}  # (h, s) -> list of et tiles

                def emit_S(h, s):
                    """S-matmuls + exp + diag masks for head h, slice s."""
                    p, half = h // 2, h % 2
                    po = half * DK
                    na = _n_alive(s, mode)
                    etl[(h, s)] = []
                    for tp2 in range(na // 2):
                        s_ps = sps.tile([128, 2 * TQ], F32,
                                        name=f"s{h}_{s}_{tp2}", tag="s")
                        # diag blocks: cols f < 128*d are masked for every
                        # partition -> skip in S/exp/mask/O. t=0 is always
                        # full width, so PSUM accumulation start covers all.
                        c0s, ds = [], []
                        for j in range(2):
                            t = 2 * tp2 + j
                            if mode == "causal" and t >= (TQ // 128) * s:
                                d = (128 * t - TQ * s) // 128
                                ds.append(d); c0s.append(128 * d)
                            else:
                                ds.append(None); c0s.append(0)
                        for j in range(2):
                            t = 2 * tp2 + j
                            c0 = c0s[j]
                            nc.tensor.matmul(
                                s_ps[:, j * TQ + c0:(j + 1) * TQ],
                                kt[p][po:po + DK, t * 128:(t + 1) * 128],
                                qt[p][po:po + DK, s * TQ + c0:(s + 1) * TQ],
                                start=True, stop=True)
                        et = etp.tile([128, 2 * TQ], BF16,
                                      name=f"et{h}_{s}_{tp2}", tag="et")
                        # One exp instruction costs ~350 extra cycles;
                        # splitting to skip dead columns only pays off when
                        # the skip is > 128 cols. For small c0 exp the dead
                        # region too (harmless: the O-matmul never reads
                        # it), starting at min(c0s).
                        if max(c0s) <= 128:
                            cm = min(c0s)
                            nc.scalar.activation(
                                et[:, cm:2 * TQ], s_ps[:, cm:2 * TQ],
                                mybir.ActivationFunctionType.Exp,
                                scale=1.0 / np.sqrt(DK))
                        else:
                            for j in range(2):
                                c0 = c0s[j]
                                nc.scalar.activation(
                                    et[:, j * TQ + c0:(j + 1) * TQ],
                                    s_ps[:, j * TQ + c0:(j + 1) * TQ],
                                    mybir.ActivationFunctionType.Exp,
                                    scale=1.0 / np.sqrt(DK))
                        for j in range(2):
                            t = 2 * tp2 + j
                            if ds[j] is not None:
                                d, c0 = ds[j], c0s[j]
                                nc.vector.tensor_mul(
                                    et[:, j * TQ + c0:j * TQ + c0 + 128],
                                    et[:, j * TQ + c0:j * TQ + c0 + 128],
                                    masks[:, d * TQ + c0:d * TQ + c0 + 128])
                            elif mode == "masked":
                                mt = mtp.tile([128, TQ], BF16,
                                              name=f"mt{h}{s}{t}", tag="mt")
                                nc.sync.dma_start(
                                    out=mt,
                                    in_=maskd[t * 128:(t + 1) * 128,
                                              s * TQ:(s + 1) * TQ])
                                nc.vector.tensor_mul(
                                    et[:, j * TQ:(j + 1) * TQ],
                                    et[:, j * TQ:(j + 1) * TQ], mt[:])
                        if DBG and h == 0 and s == 0 and tp2 == 0:
                            nc.sync.dma_start(out=dbg["det"][:], in_=et[:])
                        etl[(h, s)].append(et)
                        padv((2 * TQ - c0s[0] - c0s[1]) * PE_CY)
                        drain(PAIR_BUDGET)

                def emit_O(h, s, opool, wide=False):
                    """Flipped O-matmuls (et stationary, V_aug moving) +
                    per-q-tile normalization into stg."""
                    p, half = h // 2, h % 2
                    po = half * DK
                    force(("v", v_need(s)))
                    ets = etl.pop((h, s))
                    if wide:
                        o_ps = opool.tile([128, TQ], F32,
                                          name=f"o{h}_{s}", tag="op")
                    else:
                        o_ps = opool.tile([128, NQT * (DK + 1)], F32,
                                          name=f"o{h}_{s}", tag="o")
                    rcp = rcpp.tile([128, NQT], F32, name=f"r{h}_{s}",
                                    tag="rcp")
                    for qtl in range(NQT):
                        naq = _n_alive_qt(s, qtl, mode)
                        reg = o_ps[:, qtl * (DK + 1):(qtl + 1) * (DK + 1)]
                        for t in range(naq):
                            et = ets[t // 2]
                            j = t % 2
                            nc.tensor.matmul(
                                reg,
                                et[:, j * TQ + qtl * 128:
                                   j * TQ + (qtl + 1) * 128],
                                va[h][:, t * (DK + 1):(t + 1) * (DK + 1)],
                                start=(t == 0), stop=(t == naq - 1))
                        padv(naq * (DK + 1) * PE_CY)
                        drain(250.0)
                        if USE_DIV:
                            nc.vector.tensor_scalar(
                                out=stg[p][s][:, qtl * 128 + po:
                                              qtl * 128 + po + DK],
                                in0=reg[:, 0:DK],
                                scalar1=reg[:, DK:DK + 1],
                                scalar2=None,
                                op0=mybir.AluOpType.divide)
                    if not USE_DIV:
                        # one batched reciprocal over the 4 rowsums, then one
                        # per-partition-scalar multiply per q-tile
                        nc.vector.reciprocal(
                            rcp[:], o_ps[:, DK:NQT * (DK + 1):DK + 1])
                        for qtl in range(NQT):
                            nc.vector.tensor_scalar_mul(
                                stg[p][s][:, qtl * 128 + po:
                                          qtl * 128 + po + DK],
                                o_ps[:, qtl * (DK + 1):
                                     qtl * (DK + 1) + DK],
                                rcp[:, qtl:qtl + 1])
                    if DBG and h == 1 and s == 0:
                        nc.sync.dma_start(out=dbg["dstg00"][:],
                                          in_=stg[0][0][:])

                def emit_trans(s, m, fps):
                    """Pair transposes for q-tile m of slice s into an fps
                    slot (same byte size as the outproj psum -> shared tag),
                    then DVE copies into otu."""
                    tpt = fps.tile([128, D], BF16, name=f"tp{s}_{m}",
                                   tag="op")
                    for p in range(2):
                        nc.tensor.transpose(
                            tpt[:, p * 128:(p + 1) * 128],
                            stg[p][s][:, m * 128:(m + 1) * 128],
                            ident[:])
                    for p in range(2):
                        nc.vector.tensor_copy(
                            otu[p][s][:, m * 128:(m + 1) * 128],
                            tpt[:, p * 128:(p + 1) * 128])
                    padv(107.0)

                def emit_op(s, m, fps):
                    """Output projection for q-tile m of slice s. Staging
                    copies alternate DVE/ACT to split the load."""
                    r0 = (s * NQT + m) * 128
                    o_sb = ostp.tile([128, D], BF16, name=f"os{s}_{m}",
                                     tag="os")
                    for n in range(2):
                        o_ps2 = fps.tile([128, TQ], F32,
                                         name=f"op{s}_{m}_{n}", tag="op")
                        for p in range(2):
                            nc.tensor.matmul(
                                o_ps2[:],
                                otu[p][s][:, m * 128:(m + 1) * 128],
                                wo_s[:, p * D + n * TQ:
                                     p * D + (n + 1) * TQ],
                                start=(p == 0), stop=(p == 1))
                        if n == 0:
                            nc.vector.tensor_copy(
                                o_sb[:, n * TQ:(n + 1) * TQ], o_ps2[:])
                        else:
                            nc.scalar.copy(
                                o_sb[:, n * TQ:(n + 1) * TQ], o_ps2[:])
                        nc.sync.dma_start(
                            out=out[r0:r0 + 128, n * TQ:(n + 1) * TQ],
                            in_=o_sb[:, n * TQ:(n + 1) * TQ])
                    padv(854.0)

                def enqueue_tail(s, fps):
                    """Transpose/outproj ladder enqueued as paced fillers
                    (drained between later S-pairs); transposes run one
                    q-tile ahead of the outproj so the otu copies are off
                    the PE wait chain."""
                    q_push(lambda: emit_trans(s, 0, fps), 107.0, 0.0)
                    for m in range(NQT):
                        if m + 1 < NQT:
                            q_push(lambda m=m: emit_trans(s, m + 1, fps),
                                   107.0, 0.0)
                        q_push(lambda m=m: emit_op(s, m, fps), 854.0, 0.0)

                # ---- DMA emission + arrival estimates (DMA engines are a
                # single serial resource; the Tile scheduler preserves
                # per-engine emission order, so all overlap is hand-paced
                # with a filler queue driven by these estimates).
                DMA_LAT, SEM_LAT, FULL_T, HALF_T = 1850.0, 900.0, 1570.0, 800.0
                dma_t = [0.0]

                def dma_in(dst, src, ns):
                    nc.sync.dma_start(out=dst, in_=src)
                    dma_t[0] += ns
                    return DMA_LAT + dma_t[0] + SEM_LAT

                arr_wq = dma_in(wq_s[:], wq[:], FULL_T)
                arr_wk = dma_in(wk_s[:], wk[:], FULL_T)
                # q/k loaded in column-half waves: S(0,0) only needs token
                # columns 0:1024 of every k-tile, so attention starts ~15us
                # in instead of waiting for the full 24MB input stream.
                qTr = [xbig.tile([128, T], IN_DT, name=f"qTr{kk}",
                                 tag=f"xq{kk}") for kk in range(NKT)]
                kTr = [xbig.tile([128, T], IN_DT, name=f"kTr{kk}",
                                 tag=f"xk{kk}") for kk in range(NKT)]
                arr_qh, arr_kh = [], []
                for w in range(2):
                    lo, hi = w * (T // 2), (w + 1) * (T // 2)
                    arr_qh.append([dma_in(qTr[kk][:, lo:hi],
                                          qT[kk * 128:(kk + 1) * 128, lo:hi],
                                          HALF_T) for kk in range(NKT)])
                    arr_kh.append([dma_in(kTr[kk][:, lo:hi],
                                          kTt[kk * 128:(kk + 1) * 128, lo:hi],
                                          HALF_T) for kk in range(NKT)])
                arr_wv = dma_in(wv_s[:], wv[:], FULL_T)
                # vT loaded in two column-half waves so early V tiles land
                # before the full tensor: wave w covers key tiles 8w..8w+7
                vTr = [xbig.tile([128, T], IN_DT, name=f"vTr{kk}",
                                 tag=f"xv{kk}") for kk in range(NKT)]
                arr_vw = []
                for w in range(2):
                    for kk in range(NKT):
                        a = dma_in(vTr[kk][:, w * (T // 2):(w + 1) * (T // 2)],
                                   vT[kk * 128:(kk + 1) * 128,
                                      w * (T // 2):(w + 1) * (T // 2)],
                                   HALF_T)
                    arr_vw.append(a)
                dma_in(wo_s[:], wo[:], FULL_T)

                # ---- filler queue: (emit_fn, pe_ns, ready_ns, marker)
                import collections as _c
                queue = _c.deque()
                done = set()
                est_pe = [1500.0]
                warm_used = [0]
                wps = sps.tile([128, 2 * TQ], F32, name="wm", tag="s")

                def padv(ns):
                    est_pe[0] += ns

                def warm_one():
                    nc.tensor.matmul(wps[:, 0:128], warm[:], warm[:],
                                     start=True, stop=True)
                    warm_used[0] += 1
                    padv(55.0)

                def q_push(fn, pe_ns, ready, marker=None):
                    queue.append((fn, pe_ns, ready, marker))

                def drain(budget):
                    while budget > 0 and queue:
                        fn, pe_ns, ready, mk = queue[0]
                        if ready > est_pe[0] + 150:
                            break
                        queue.popleft()
                        fn()
                        est_pe[0] = max(est_pe[0], ready) + pe_ns
                        budget -= pe_ns
                        if mk:
                            done.add(mk)

                def force(marker):
                    while marker not in done and queue:
                        fn, pe_ns, ready, mk = queue[0]
                        if ready > est_pe[0] + 150 and \
                                warm_used[0] < N_WARMUP:
                            warm_one()
                            continue
                        queue.popleft()
                        fn()
                        est_pe[0] = max(est_pe[0], ready) + pe_ns
                        if mk:
                            done.add(mk)

                def drain_all():
                    while queue:
                        fn, pe_ns, ready, mk = queue.popleft()
                        fn()
                        est_pe[0] = max(est_pe[0], ready) + pe_ns
                        if mk:
                            done.add(mk)

                PJ = [None, "qk"]  # current proj psum pool + tag

                def proj_fillers(xr, w_s, dst, m, nA, nB, readys, mk):
                    tiles = {}

                    def mkf(n, kk):
                        def f():
                            if kk == 0:
                                tiles[n] = PJ[0].tile(
                                    [128, TQ], F32, name=f"pj{mk}{n}",
                                    tag=PJ[1])
                            nc.tensor.matmul(
                                tiles[n][:],
                                w_s[:, kk * G + m * 128:
                                    kk * G + (m + 1) * 128],
                                xr[kk][:, n * TQ:(n + 1) * TQ],
                                start=(kk == 0), stop=(kk == NKT - 1))
                            if kk == NKT - 1:
                                nc.vector.tensor_copy(
                                    dst[m][:, n * TQ:(n + 1) * TQ],
                                    tiles[n][:])
                        return f
                    for kk in range(NKT):
                        for n in dict.fromkeys((nA, nB)):
                            q_push(mkf(n, kk), 216.0, readys(n, kk),
                                   marker=(mk, n) if kk == NKT - 1 else None)

                def v_fillers():
                    cur = {}

                    def mkf(t, kk):
                        def f():
                            if kk == 0:
                                cur[t] = qkps.tile([128, TQ], F32,
                                                   name=f"v{t}", tag="qk")
                            nc.tensor.matmul(
                                cur[t][:, 0:G],
                                vTr[kk][:, t * 128:(t + 1) * 128],
                                wv_s[:, kk * G:(kk + 1) * G],
                                start=(kk == 0), stop=(kk == NKT - 1))
                            if kk == NKT - 1:
                                for h in range(HG):
                                    nc.vector.tensor_copy(
                                        va[h][:, t * (DK + 1):
                                              t * (DK + 1) + DK],
                                        cur[t][:, h * DK:(h + 1) * DK])
                        return f
                    for t in range(NT):
                        rdy = max(arr_wv, arr_vw[t // 8])
                        for kk in range(NKT):
                            q_push(mkf(t, kk), 110.0, rdy,
                                   marker=("v", t) if kk == NKT - 1 else None)

                qkps_cm = tc.tile_pool(name="qkps", bufs=2, space="PSUM")
                qkps = qkps_cm.__enter__()
                PJ[0] = qkps
                oap_cm = tc.tile_pool(name="oap", bufs=2, space="PSUM")
                oap = oap_cm.__enter__()
                rq = lambda n, kk: max(arr_wq, arr_qh[n // 2][kk])
                rk = lambda n, kk: max(arr_wk, arr_kh[n // 2][kk])
                proj_fillers(qTr, wq_s, qt, 0, 0, 1, rq, "q0")
                proj_fillers(kTr, wk_s, kt, 0, 0, 1, rk, "k0")
                proj_fillers(qTr, wq_s, qt, 0, 2, 3, rq, "q0")
                proj_fillers(kTr, wk_s, kt, 0, 2, 3, rk, "k0")
                v_fillers()
                # m=1 groups n=3,2 drain through head 1 (deadline: merged
                # phase entry runs slices largest-first); n=1,0 are enqueued
                # inside the merged phase as its i=0 filler.
                proj_fillers(qTr, wq_s, qt, 1, 3, 2, rq, "q1")
                proj_fillers(kTr, wk_s, kt, 1, 3, 2, rk, "k1")

                def s_need(s):
                    return s if mode == "causal" else NS - 1

                def v_need(s):
                    return _n_alive(s, mode) - 1

                # heads 0-1 (qkps pool open for proj + V psum); all m1
                # projections must drain by the end of head 1 (head 1 has
                # PE slack: its attention is ACT-bound). Head 1 runs its
                # slices largest-first so later heads can too.
                for s in range(NS):
                    force(("k0", s_need(s)))
                    emit_S(0, s)
                    if s > 0:
                        emit_O(0, s - 1, oap)
                desc = list(range(NS - 1, -1, -1))
                for i, s in enumerate(desc):
                    emit_S(1, s)
                    emit_O(*((0, NS - 1) if i == 0 else (1, desc[i - 1])),
                           oap)
                emit_O(1, desc[-1], oap)
                drain_all()
                oap_cm.__exit__(None, None, None)
                qkps_cm.__exit__(None, None, None)
                # heads 2+3 merged, slices largest-first, with the
                # transpose/outproj tail enqueued as the PE filler: the
                # phase stays PE-bound (S+O+tail vs two heads' exps),
                # absorbing per-slice exp stalls, and the final serial
                # cascade is the smallest slice. O-psums, transposes and
                # outproj psums share one 4-slot rotating pool so WAR
                # waits sit 4 requests back and copy latencies hide.
                morder = [0, NS - 1, NS - 2, NS - 3]
                with tc.tile_pool(name="fps", bufs=4, space="PSUM") as fps:
                    PJ[0], PJ[1] = fps, "op"
                    # solo groups, n=0 first: force at i=0 pulls only n=0;
                    # the n=1 groups stay queued as filler for the big
                    # slice-3 iteration (deadline: the last iteration)
                    proj_fillers(qTr, wq_s, qt, 1, 0, 0, rq, "q1")
                    proj_fillers(kTr, wk_s, kt, 1, 0, 0, rk, "k1")
                    proj_fillers(qTr, wq_s, qt, 1, 1, 1, rq, "q1")
                    proj_fillers(kTr, wk_s, kt, 1, 1, 1, rk, "k1")
                    for i, s in enumerate(morder):
                        force(("k1", s if mode == "causal" else 1))
                        emit_S(2, s)
                        if i > 0:
                            emit_O(3, morder[i - 1], fps, wide=True)
                        emit_S(3, s)
                        emit_O(2, s, fps, wide=True)
                        if i > 0:
                            enqueue_tail(morder[i - 1], fps)
                    emit_O(3, morder[-1], fps, wide=True)
                    enqueue_tail(morder[-1], fps)
                    drain_all()

            if DBG:
                nc.sync.dma_start(out=dbg["dqt0"][:], in_=qt[0][:])
                nc.sync.dma_start(out=dbg["dkt0"][:], in_=kt[0][:])
                nc.sync.dma_start(out=dbg["dva0"][:], in_=va[0][:])
                nc.sync.dma_start(out=dbg["dotu00"][:], in_=otu[0][0][:])

            otrs_cm.__exit__(None, None, None)

    split_multi_waits(nc)
    return nc


def _detect_mode(mask):
    if mask.all():
        return "dense"
    if np.array_equal(mask, np.tril(np.ones((T, T), dtype=bool))):
        return "causal"
    return "masked"


def kernel(q, k, v, mask, Wq, Wk, Wv, Wo, _trace=False, _trace_kwargs=None):
    q, k, v = np.asarray(q), np.asarray(k), np.asarray(v)
    Wq, Wk, Wv, Wo = (np.asarray(Wq), np.asarray(Wk),
                      np.asarray(Wv), np.asarray(Wo))
    mask = np.asarray(mask)
    mode = _detect_mode(mask)
    nc = build_program(mode)

    in_maps = []
    for c in range(8):
        b, g = c // 4, c % 4
        bf = ml_dtypes.bfloat16

        def packw(w):  # [D, G] -> [128, NKT*G] SBUF layout
            return np.ascontiguousarray(
                w.reshape(NKT, 128, G).transpose(1, 0, 2).reshape(128, NKT * G)
                .astype(bf))

        wo_sl = Wo[g * G:(g + 1) * G, :]
        im = {
            "qT": np.ascontiguousarray(q[b].T.astype(bf)),
            "kT": np.ascontiguousarray(k[b].T.astype(bf)),
            "vT": np.ascontiguousarray(v[b].T.astype(bf)),
            "wq": packw(Wq[:, g * G:(g + 1) * G]),
            "wk": packw(Wk[:, g * G:(g + 1) * G]),
            "wv": packw(Wv[:, g * G:(g + 1) * G]),
            "wo": np.ascontiguousarray(
                wo_sl.reshape(2, 128, D).transpose(1, 0, 2).reshape(128, 2 * D)
                .astype(bf)),
        }
        if mode == "masked":
            im["maskT"] = np.ascontiguousarray(
                mask.T.astype(ml_dtypes.bfloat16))
        in_maps.append(im)

    res = run_bass_kernel_spmd(nc, in_maps, list(range(8)), trace=_trace,
                               **(_trace_kwargs or {}))
    outs = [np.asarray(res.results[c]["out"]).astype(np.float32)
            for c in range(8)]
    full = np.stack([outs[4 * b] + outs[4 * b + 1] + outs[4 * b + 2]
                     + outs[4 * b + 3] for b in range(B)])
    if _trace:
        return full, res
    return full


# revision 4
# speedup vs baseline: 1.1321x; 1.0360x over previous
"""Multi-head causal attention (B=2, T=2048, D=1024, H=16, dk=dv=64) on 8 NeuronCores.

Sharding: data parallel over batch (2) x tensor parallel over heads (4 groups of 4).
Core c handles batch c//4, heads [4*(c%4), 4*(c%4)+4). Each core computes the
partial output sum over its 4 heads; host adds the 4 partials per batch.

Per-core pipeline (the O-matmul uses et as STATIONARY and V_aug [128 keys, 65]
as MOVING -> 65 moving-cols per (key-tile x query-tile) instead of 512):
  QT/KT [256, T] = W.T @ xT        (bf16 matmuls, PSUM k-accumulation)
  V_aug [T, 65/head] (65th col = ones) via direct [T-part, G] projection
  per head, per tq-slice (512), per tk-tile-pair (2x128), causal-skipped:
    ST block [tk 128, tq 512] = KT_h-slice @ QT_h      (K=64)
    ET = exp(0.125 * ST)  (ACT, PSUM->SBUF bf16, batched over 2 blocks)
    diag blocks: ET *= 0/1 mask (DVE)
  per q-tile (128): O_aug [128 q, 65] += et-slice.T @ V_aug_h  (M=65 matmuls,
    et stationary); O-block lags the S-block by one slice to hide exp latency
    normalize: stg = O[:, 0:64] * recip(O[:, 64]) (per-partition scalar)
  per pair, q-tile: PE-transpose stg [q, dv2] -> otu [dv2, q]
  out [T, 1024] = otu.T @ Wo  (partial over this core's 4 heads, bf16 out)
"""
import sys

sys.path.insert(0, "/opt/trn_rl_repo")

import functools
import os
import ml_dtypes
import numpy as np

import concourse.bass as bass
import concourse.tile as tile
from concourse import mybir
from concourse.bass_utils import run_bass_kernel_spmd
from concourse.masks import make_identity

B, T, D = 2, 2048, 1024
H, DK = 16, 64            # total heads
HG = 4                    # heads per core
G = HG * DK               # 256: per-core column group width
NKT = D // 128            # 8 k-tiles of the model dim
NT = T // 128             # 16 tk tiles
NS = 4                    # tq slices
TQ = T // NS              # 512
NQT = TQ // 128           # 4 q-tiles per slice
F32 = mybir.dt.float32
BF16 = mybir.dt.bfloat16
IN_DT = BF16  # dtype for x / Wq / Wk / Wv (projection operands)
N_WARMUP = int(os.environ.get("KWARM", "400"))      # cap on filler warmups
USE_DIV = bool(int(os.environ.get("KDIV", "0")))
PAIR_BUDGET = float(os.environ.get("KPB", "250"))   # filler ns per S-pair
PE_CY = 1.0 / 2.4                                   # ns per PE cycle (ramped)


def split_multi_waits(nc, max_waits=1):
    """This walrus build has tiny per-instruction sync-wait slot limits (1 for
    matmul LW, ~2 for CTRL). Move excess waits onto preceding same-engine
    NOPs - identical semantics since each engine executes serially."""
    for func in nc.m.functions:
        for bb in func.blocks:
            out = []
            for inst in list(bb.instructions):
                si = inst.sync_info
                waits = list(si.on_wait) if (si and si.on_wait) else []
                if len(waits) > max_waits:
                    extra, keep = waits[:-max_waits], waits[-max_waits:]
                    for j, w in enumerate(extra):
                        nop = mybir.InstNoOp(name=f"{inst.name}-ws{j}")
                        nop.engine = inst.engine
                        nop.sync_info = mybir.SyncInfo(on_wait=[w], on_update=[])
                        out.append(nop)
                    inst.sync_info = mybir.SyncInfo(
                        on_wait=keep, on_update=list(si.on_update or []))
                out.append(inst)
            bb.instructions = out


def _n_alive(s, mode):
    """Number of tk tiles needed for tq slice s."""
    return NT if mode != "causal" else (TQ // 128) * (s + 1)


def _n_alive_qt(s, qt, mode):
    """Number of tk tiles needed for q-tile qt of slice s (128-granular)."""
    return NT if mode != "causal" else (TQ // 128) * s + qt + 1


@functools.lru_cache(maxsize=4)
def build_program(mode, _env=None):
    assert mode in ("causal", "dense", "masked")
    nc = bass.Bass()
    qT = nc.dram_tensor("qT", [D, T], IN_DT, kind="ExternalInput")
    kTt = nc.dram_tensor("kT", [D, T], IN_DT, kind="ExternalInput")
    vT = nc.dram_tensor("vT", [D, T], IN_DT, kind="ExternalInput")
    # weights pre-packed on host into SBUF layout: [128, NKT*G] with
    # partition p holding wq[kk*128+p, :] at cols [kk*G, (kk+1)*G)
    wq = nc.dram_tensor("wq", [128, NKT * G], IN_DT, kind="ExternalInput")
    wk = nc.dram_tensor("wk", [128, NKT * G], IN_DT, kind="ExternalInput")
    wv = nc.dram_tensor("wv", [128, NKT * G], IN_DT, kind="ExternalInput")
    wo = nc.dram_tensor("wo", [128, 2 * D], BF16, kind="ExternalInput")
    out = nc.dram_tensor("out", [T, D], BF16, kind="ExternalOutput")
    DBG = bool(int(os.environ.get("KDBG", "0")))
    dbg = {}
    if DBG:
        for nm, shape, dt_ in [("dqt0", [128, T], BF16),
                               ("dkt0", [128, T], BF16),
                               ("dva0", [128, NT * (DK + 1)], BF16),
                               ("dstg00", [128, NQT * 128], BF16),
                               ("dotu00", [128, TQ], BF16),
                               ("det", [128, 2 * TQ], BF16)]:
            dbg[nm] = nc.dram_tensor(nm, shape, dt_, kind="ExternalOutput")
    maskd = None
    if mode == "masked":
        maskd = nc.dram_tensor("maskT", [T, T], BF16, kind="ExternalInput")

    with tile.TileContext(nc) as tc:
        with (
            tc.tile_pool(name="sing", bufs=1) as sing,
            tc.tile_pool(name="xbig", bufs=1) as xbig,
            tc.tile_pool(name="etp", bufs=18) as etp,
            tc.tile_pool(name="ost", bufs=4) as ostp,
            tc.tile_pool(name="rcpp", bufs=4) as rcpp,
        ):
            # ---------------- constants ----------------
            wq_s = sing.tile([128, NKT * G], IN_DT)
            wk_s = sing.tile([128, NKT * G], IN_DT)
            wv_s = sing.tile([128, NKT * G], IN_DT)
            # DMA issue order follows need-time: wv before vTr (V-matmuls
            # consume both first); wq/wk after vTr; wo last (outproj only).
            nc.sync.dma_start(out=wv_s[:], in_=wv[:])
            wo_s = sing.tile([128, 2 * D], BF16)
            ones_sb = sing.tile([128, NT], BF16)
            nc.vector.memset(ones_sb[:], 1.0)
            ident = sing.tile([128, 128], BF16)
            make_identity(nc, ident[:])
            warm = sing.tile([128, 128], BF16)
            nc.vector.memset(warm[:], 0.0)
            qt = [sing.tile([128, T], BF16, name=f"qt{p}") for p in range(2)]
            kt = [sing.tile([128, T], BF16, name=f"kt{p}") for p in range(2)]
            va = [sing.tile([128, NT * (DK + 1)], BF16, name=f"va{h}")
                  for h in range(HG)]
            # aug-last: ones col at 64 of each 65-wide group (rowsum row)
            for h in range(HG):
                nc.vector.tensor_copy(va[h][:, DK::DK + 1], ones_sb[:])
            # normalized per-(pair, slice) outputs [q, dv-pair], q-tile major
            stg = [[sing.tile([128, NQT * 128], BF16, name=f"stg{p}_{s}")
                    for s in range(NS)] for p in range(2)]

            # ------- fused projections + attention (single scheduling region)
            otrs_cm = tc.tile_pool(name="otrs", bufs=1)
            otrs = otrs_cm.__enter__()
            otu = [[otrs.tile([128, TQ], BF16, name=f"otu{p}_{s}")
                    for s in range(NS)] for p in range(2)]
            with nc.named_scope("attn"), \
                 tc.tile_pool(name="sps", bufs=2, space="PSUM") as sps, \
                 tc.tile_pool(name="mtp", bufs=4) as mtp:
                etl = {}  # (h, s) -> list of et tiles

                def emit_S(h, s):
                    """S-matmuls + exp + diag masks for head h, slice s."""
                    p, half = h // 2, h % 2
                    po = half * DK
                    na = _n_alive(s, mode)
                    etl[(h, s)] = []
                    for tp2 in range(na // 2):
                        s_ps = sps.tile([128, 2 * TQ], F32,
                                        name=f"s{h}_{s}_{tp2}", tag="s")
                        # diag blocks: cols f < 128*d are masked for every
                        # partition -> skip in S/exp/mask/O. t=0 is always
                        # full width, so PSUM accumulation start covers all.
                        c0s, ds = [], []
                        for j in range(2):
                            t = 2 * tp2 + j
                            if mode == "causal" and t >= (TQ // 128) * s:
                                d = (128 * t - TQ * s) // 128
                                ds.append(d); c0s.append(128 * d)
                            else:
                                ds.append(None); c0s.append(0)
                        for j in range(2):
                            t = 2 * tp2 + j
                            c0 = c0s[j]
                            nc.tensor.matmul(
                                s_ps[:, j * TQ + c0:(j + 1) * TQ],
                                kt[p][po:po + DK, t * 128:(t + 1) * 128],
                                qt[p][po:po + DK, s * TQ + c0:(s + 1) * TQ],
                                start=True, stop=True)
                        et = etp.tile([128, 2 * TQ], BF16,
                                      name=f"et{h}_{s}_{tp2}", tag="et")
                        # One exp instruction costs ~350 extra cycles;
                        # splitting to skip dead columns only pays off when
                        # the skip is > 128 cols. For small c0 exp the dead
                        # region too (harmless: the O-matmul never reads
                        # it), starting at min(c0s).
                        if max(c0s) <= 128:
                            cm = min(c0s)
                            nc.scalar.activation(
                                et[:, cm:2 * TQ], s_ps[:, cm:2 * TQ],
                                mybir.ActivationFunctionType.Exp,
                                scale=1.0 / np.sqrt(DK))
                        else:
                            for j in range(2):
                                c0 = c0s[j]
                                nc.scalar.activation(
                                    et[:, j * TQ + c0:(j + 1) * TQ],
                                    s_ps[:, j * TQ + c0:(j + 1) * TQ],
                                    mybir.ActivationFunctionType.Exp,
                                    scale=1.0 / np.sqrt(DK))
                        for j in range(2):
                            t = 2 * tp2 + j
                            if ds[j] is not None:
                                c0 = c0s[j]
                                # causal diag block: zero cols < row, on the
                                # otherwise-idle Pool engine (keeps DVE off
                                # the exp->mask->O critical chain)
                                nc.gpsimd.affine_select(
                                    out=et[:, j * TQ + c0:j * TQ + c0 + 128],
                                    in_=et[:, j * TQ + c0:j * TQ + c0 + 128],
                                    compare_op=mybir.AluOpType.is_ge,
                                    fill=0.0, base=0, channel_multiplier=-1,
                                    pattern=[[1, 128]])
                            elif mode == "masked":
                                mt = mtp.tile([128, TQ], BF16,
                                              name=f"mt{h}{s}{t}", tag="mt")
                                nc.sync.dma_start(
                                    out=mt,
                                    in_=maskd[t * 128:(t + 1) * 128,
                                              s * TQ:(s + 1) * TQ])
                                nc.vector.tensor_mul(
                                    et[:, j * TQ:(j + 1) * TQ],
                                    et[:, j * TQ:(j + 1) * TQ], mt[:])
                        if DBG and h == 0 and s == 0 and tp2 == 0:
                            nc.sync.dma_start(out=dbg["det"][:], in_=et[:])
                        etl[(h, s)].append(et)
                        padv((2 * TQ - c0s[0] - c0s[1]) * PE_CY)
                        drain(PAIR_BUDGET)

                def emit_O(h, s, opool, wide=False, norm_act=False):
                    """Flipped O-matmuls (et stationary, V_aug moving) +
                    per-q-tile normalization into stg."""
                    p, half = h // 2, h % 2
                    po = half * DK
                    force(("v", v_need(s)))
                    ets = etl.pop((h, s))
                    if wide:
                        o_ps = opool.tile([128, TQ], F32,
                                          name=f"o{h}_{s}", tag="op")
                    else:
                        o_ps = opool.tile([128, NQT * (DK + 1)], F32,
                                          name=f"o{h}_{s}", tag="o")
                    rcp = rcpp.tile([128, NQT], F32, name=f"r{h}_{s}",
                                    tag="rcp")
                    for qtl in range(NQT):
                        naq = _n_alive_qt(s, qtl, mode)
                        reg = o_ps[:, qtl * (DK + 1):(qtl + 1) * (DK + 1)]
                        for t in range(naq):
                            et = ets[t // 2]
                            j = t % 2
                            nc.tensor.matmul(
                                reg,
                                et[:, j * TQ + qtl * 128:
                                   j * TQ + (qtl + 1) * 128],
                                va[h][:, t * (DK + 1):(t + 1) * (DK + 1)],
                                start=(t == 0), stop=(t == naq - 1))
                        padv(naq * (DK + 1) * PE_CY)
                        drain(250.0)
                        if USE_DIV:
                            nc.vector.tensor_scalar(
                                out=stg[p][s][:, qtl * 128 + po:
                                              qtl * 128 + po + DK],
                                in0=reg[:, 0:DK],
                                scalar1=reg[:, DK:DK + 1],
                                scalar2=None,
                                op0=mybir.AluOpType.divide)
                    if not USE_DIV:
                        # one batched reciprocal over the 4 rowsums, then one
                        # per-partition-scalar multiply per q-tile (on ACT
                        # for the endgame slices, where the exp stream has
                        # drained and DVE is the congested engine)
                        nc.vector.reciprocal(
                            rcp[:], o_ps[:, DK:NQT * (DK + 1):DK + 1])
                        for qtl in range(NQT):
                            if norm_act:
                                nc.scalar.activation(
                                    stg[p][s][:, qtl * 128 + po:
                                              qtl * 128 + po + DK],
                                    o_ps[:, qtl * (DK + 1):
                                         qtl * (DK + 1) + DK],
                                    mybir.ActivationFunctionType.Copy,
                                    scale=rcp[:, qtl:qtl + 1])
                            else:
                                nc.vector.tensor_scalar_mul(
                                    stg[p][s][:, qtl * 128 + po:
                                              qtl * 128 + po + DK],
                                    o_ps[:, qtl * (DK + 1):
                                         qtl * (DK + 1) + DK],
                                    rcp[:, qtl:qtl + 1])
                    if DBG and h == 1 and s == 0:
                        nc.sync.dma_start(out=dbg["dstg00"][:],
                                          in_=stg[0][0][:])

                def emit_trans(s, m, fps, act=False):
                    """Pair transposes for q-tile m of slice s into an fps
                    slot (same byte size as the outproj psum -> shared tag),
                    then copies into otu (ACT for the final tail, where
                    the exp stream has drained and ACT idles)."""
                    tpt = fps.tile([128, D], BF16, name=f"tp{s}_{m}",
                                   tag="op")
                    for p in range(2):
                        nc.tensor.transpose(
                            tpt[:, p * 128:(p + 1) * 128],
                            stg[p][s][:, m * 128:(m + 1) * 128],
                            ident[:])
                    for p in range(2):
                        eng = nc.scalar if (act and p == 1) else nc.vector
                        if eng is nc.scalar:
                            nc.scalar.copy(
                                otu[p][s][:, m * 128:(m + 1) * 128],
                                tpt[:, p * 128:(p + 1) * 128])
                        else:
                            nc.vector.tensor_copy(
                                otu[p][s][:, m * 128:(m + 1) * 128],
                                tpt[:, p * 128:(p + 1) * 128])
                    padv(107.0)

                def emit_op(s, m, fps, act=False):
                    """Output projection for q-tile m of slice s. The final
                    tail's staging copies go to the otherwise-idle ACT."""
                    r0 = (s * NQT + m) * 128
                    o_sb = ostp.tile([128, D], BF16, name=f"os{s}_{m}",
                                     tag="os")
                    for n in range(2):
                        o_ps2 = fps.tile([128, TQ], F32,
                                         name=f"op{s}_{m}_{n}", tag="op")
                        for p in range(2):
                            nc.tensor.matmul(
                                o_ps2[:],
                                otu[p][s][:, m * 128:(m + 1) * 128],
                                wo_s[:, p * D + n * TQ:
                                     p * D + (n + 1) * TQ],
                                start=(p == 0), stop=(p == 1))
                        if act and n == 1:
                            nc.scalar.copy(
                                o_sb[:, n * TQ:(n + 1) * TQ], o_ps2[:])
                        else:
                            nc.vector.tensor_copy(
                                o_sb[:, n * TQ:(n + 1) * TQ], o_ps2[:])
                        nc.sync.dma_start(
                            out=out[r0:r0 + 128, n * TQ:(n + 1) * TQ],
                            in_=o_sb[:, n * TQ:(n + 1) * TQ])
                    padv(854.0)

                def enqueue_tail(s, fps, act=False):
                    """Transpose/outproj ladder enqueued as paced fillers
                    (drained between later S-pairs); transposes run one
                    q-tile ahead of the outproj so the otu copies are off
                    the PE wait chain."""
                    q_push(lambda: emit_trans(s, 0, fps, act), 107.0, 0.0)
                    for m in range(NQT):
                        if m + 1 < NQT:
                            q_push(lambda m=m: emit_trans(s, m + 1, fps,
                                                          act),
                                   107.0, 0.0)
                        q_push(lambda m=m: emit_op(s, m, fps, act),
                               854.0, 0.0)

                # ---- DMA emission + arrival estimates (DMA engines are a
                # single serial resource; the Tile scheduler preserves
                # per-engine emission order, so all overlap is hand-paced
                # with a filler queue driven by these estimates).
                DMA_LAT, SEM_LAT, FULL_T, HALF_T = 1850.0, 900.0, 1570.0, 800.0
                dma_t = [0.0]

                def dma_in(dst, src, ns):
                    nc.sync.dma_start(out=dst, in_=src)
                    dma_t[0] += ns
                    return DMA_LAT + dma_t[0] + SEM_LAT

                arr_wq = dma_in(wq_s[:], wq[:], FULL_T)
                arr_wk = dma_in(wk_s[:], wk[:], FULL_T)
                # q/k/v loaded in column-half waves interleaved so useful
                # work lands as early as possible: S(·,0/1) + all n<=1
                # projection groups need only columns 0:1024 of q/k, and
                # V tiles t<8 need only columns 0:1024 of v.
                qTr = [xbig.tile([128, T], IN_DT, name=f"qTr{kk}",
                                 tag=f"xq{kk}") for kk in range(NKT)]
                kTr = [xbig.tile([128, T], IN_DT, name=f"kTr{kk}",
                                 tag=f"xk{kk}") for kk in range(NKT)]
                vTr = [xbig.tile([128, T], IN_DT, name=f"vTr{kk}",
                                 tag=f"xv{kk}") for kk in range(NKT)]

                def half_wave(tiles, src, w):
                    lo, hi = w * (T // 2), (w + 1) * (T // 2)
                    return [dma_in(tiles[kk][:, lo:hi],
                                   src[kk * 128:(kk + 1) * 128, lo:hi],
                                   HALF_T) for kk in range(NKT)]

                arr_qh, arr_kh, arr_vw = [], [], []
                arr_qh.append(half_wave(qTr, qT, 0))
                arr_kh.append(half_wave(kTr, kTt, 0))
                arr_wv = dma_in(wv_s[:], wv[:], FULL_T)
                arr_vw.append(half_wave(vTr, vT, 0)[-1])
                arr_qh.append(half_wave(qTr, qT, 1))
                arr_kh.append(half_wave(kTr, kTt, 1))
                arr_vw.append(half_wave(vTr, vT, 1)[-1])
                dma_in(wo_s[:], wo[:], FULL_T)

                # ---- filler queue: (emit_fn, pe_ns, ready_ns, marker)
                import collections as _c
                queue = _c.deque()
                done = set()
                est_pe = [1500.0]
                warm_used = [0]
                wps = sps.tile([128, 2 * TQ], F32, name="wm", tag="s")

                def padv(ns):
                    est_pe[0] += ns

                def warm_one():
                    nc.tensor.matmul(wps[:, 0:128], warm[:], warm[:],
                                     start=True, stop=True)
                    warm_used[0] += 1
                    padv(55.0)

                def q_push(fn, pe_ns, ready, marker=None):
                    queue.append((fn, pe_ns, ready, marker))

                def drain(budget):
                    while budget > 0 and queue:
                        fn, pe_ns, ready, mk = queue[0]
                        if ready > est_pe[0] + 150:
                            break
                        queue.popleft()
                        fn()
                        est_pe[0] = max(est_pe[0], ready) + pe_ns
                        budget -= pe_ns
                        if mk:
                            done.add(mk)

                def force(marker):
                    while marker not in done and queue:
                        fn, pe_ns, ready, mk = queue[0]
                        if ready > est_pe[0] + 150 and \
                                warm_used[0] < N_WARMUP:
                            warm_one()
                            continue
                        queue.popleft()
                        fn()
                        est_pe[0] = max(est_pe[0], ready) + pe_ns
                        if mk:
                            done.add(mk)

                def drain_all():
                    while queue:
                        fn, pe_ns, ready, mk = queue.popleft()
                        fn()
                        est_pe[0] = max(est_pe[0], ready) + pe_ns
                        if mk:
                            done.add(mk)

                PJ = [None, "qk"]  # current proj psum pool + tag

                def proj_fillers(xr, w_s, dst, m, nA, nB, readys, mk):
                    tiles = {}

                    def mkf(n, kk):
                        def f():
                            if kk == 0:
                                tiles[n] = PJ[0].tile(
                                    [128, TQ], F32, name=f"pj{mk}{n}",
                                    tag=PJ[1])
                            nc.tensor.matmul(
                                tiles[n][:],
                                w_s[:, kk * G + m * 128:
                                    kk * G + (m + 1) * 128],
                                xr[kk][:, n * TQ:(n + 1) * TQ],
                                start=(kk == 0), stop=(kk == NKT - 1))
                            if kk == NKT - 1:
                                nc.vector.tensor_copy(
                                    dst[m][:, n * TQ:(n + 1) * TQ],
                                    tiles[n][:])
                        return f
                    for kk in range(NKT):
                        for n in dict.fromkeys((nA, nB)):
                            q_push(mkf(n, kk), 216.0, readys(n, kk),
                                   marker=(mk, n) if kk == NKT - 1 else None)

                def v_fillers(ts):
                    cur = {}

                    def mkf(t, kk):
                        def f():
                            if kk == 0:
                                cur[t] = qkps.tile([128, TQ], F32,
                                                   name=f"v{t}", tag="qk")
                            nc.tensor.matmul(
                                cur[t][:, 0:G],
                                vTr[kk][:, t * 128:(t + 1) * 128],
                                wv_s[:, kk * G:(kk + 1) * G],
                                start=(kk == 0), stop=(kk == NKT - 1))
                            if kk == NKT - 1:
                                for h in range(HG):
                                    nc.vector.tensor_copy(
                                        va[h][:, t * (DK + 1):
                                              t * (DK + 1) + DK],
                                        cur[t][:, h * DK:(h + 1) * DK])
                        return f
                    for t in ts:
                        rdy = max(arr_wv, arr_vw[t // (NT // 2)])
                        for kk in range(NKT):
                            q_push(mkf(t, kk), 110.0, rdy,
                                   marker=("v", t) if kk == NKT - 1 else None)

                qkps_cm = tc.tile_pool(name="qkps", bufs=2, space="PSUM")
                qkps = qkps_cm.__enter__()
                PJ[0] = qkps
                oap_cm = tc.tile_pool(name="oap", bufs=2, space="PSUM")
                oap = oap_cm.__enter__()
                rq = lambda n, kk: max(arr_wq, arr_qh[n // 2][kk])
                rk = lambda n, kk: max(arr_wk, arr_kh[n // 2][kk])
                # half-0-dependent groups first (m0 and m1 n<=1), then V
                # t<8, then the half-1 groups; m1 n=3,2 defer to phase B.
                proj_fillers(qTr, wq_s, qt, 0, 0, 1, rq, "q0")
                proj_fillers(kTr, wk_s, kt, 0, 0, 1, rk, "k0")
                proj_fillers(qTr, wq_s, qt, 1, 0, 1, rq, "q1")
                proj_fillers(kTr, wk_s, kt, 1, 0, 1, rk, "k1")
                v_fillers(range(NT // 2))
                proj_fillers(qTr, wq_s, qt, 0, 2, 3, rq, "q0")
                proj_fillers(kTr, wk_s, kt, 0, 2, 3, rk, "k0")
                v_fillers(range(NT // 2, NT))

                def s_need(s):
                    return s if mode == "causal" else NS - 1

                def v_need(s):
                    return _n_alive(s, mode) - 1

                # phase A: heads 0+1 interleaved, ascending slices (slice s
                # needs only the n<=s projection groups; the filler queue
                # supplies m0/m1 projections and the V projection, paced
                # against the DMA wave arrivals)
                for s in range(NS):
                    if s == NS - 1:
                        # m1 n=3,2 projections: filler for the big final
                        # slice of this phase (PE-idle while ACT drains
                        # both heads' exps); must complete before the qkps
                        # pool closes (drain_all below)
                        proj_fillers(qTr, wq_s, qt, 1, 3, 2, rq, "q1")
                        proj_fillers(kTr, wk_s, kt, 1, 3, 2, rk, "k1")
                    force(("k0", s_need(s)))
                    emit_S(0, s)
                    if s > 0:
                        emit_O(1, s - 1, oap)
                    emit_S(1, s)
                    emit_O(0, s, oap)
                emit_O(1, NS - 1, oap)
                drain_all()
                oap_cm.__exit__(None, None, None)
                qkps_cm.__exit__(None, None, None)
                # heads 2+3 merged, slices largest-first, with the
                # transpose/outproj tail enqueued as the PE filler: the
                # phase stays PE-bound (S+O+tail vs two heads' exps),
                # absorbing per-slice exp stalls, and the final serial
                # cascade is the smallest slice. O-psums, transposes and
                # outproj psums share one 4-slot rotating pool so WAR
                # waits sit 4 requests back and copy latencies hide.
                morder = [1, NS - 1, NS - 2, 0] if NS == 4 \
                    else list(range(NS))
                with tc.tile_pool(name="fps", bufs=4, space="PSUM") as fps:
                    PJ[0], PJ[1] = fps, "op"
                    last = len(morder) - 1
                    for i, s in enumerate(morder):
                        force(("k1", s if mode == "causal" else 2))
                        emit_S(2, s)
                        if i > 0:
                            emit_O(3, morder[i - 1], fps, wide=True)
                        emit_S(3, s)
                        emit_O(2, s, fps, wide=True, norm_act=(i == last))
                        if i > 0:
                            enqueue_tail(morder[i - 1], fps,
                                         act=(i == last))
                    emit_O(3, morder[-1], fps, wide=True, norm_act=True)
                    enqueue_tail(morder[-1], fps, act=True)
                    drain_all()

            if DBG:
                nc.sync.dma_start(out=dbg["dqt0"][:], in_=qt[0][:])
                nc.sync.dma_start(out=dbg["dkt0"][:], in_=kt[0][:])
                nc.sync.dma_start(out=dbg["dva0"][:], in_=va[0][:])
                nc.sync.dma_start(out=dbg["dotu00"][:], in_=otu[0][0][:])

            otrs_cm.__exit__(None, None, None)

    split_multi_waits(nc)
    return nc


def _detect_mode(mask):
    if mask.all():
        return "dense"
    if np.array_equal(mask, np.tril(np.ones((T, T), dtype=bool))):
        return "causal"
    return "masked"


def kernel(q, k, v, mask, Wq, Wk, Wv, Wo, _trace=False, _trace_kwargs=None):
    q, k, v = np.asarray(q), np.asarray(k), np.asarray(v)
    Wq, Wk, Wv, Wo = (np.asarray(Wq), np.asarray(Wk),
                      np.asarray(Wv), np.asarray(Wo))
    mask = np.asarray(mask)
    mode = _detect_mode(mask)
    nc = build_program(mode)

    in_maps = []
    for c in range(8):
        b, g = c // 4, c % 4
        bf = ml_dtypes.bfloat16

        def packw(w):  # [D, G] -> [128, NKT*G] SBUF layout
            return np.ascontiguousarray(
                w.reshape(NKT, 128, G).transpose(1, 0, 2).reshape(128, NKT * G)
                .astype(bf))

        wo_sl = Wo[g * G:(g + 1) * G, :]
        im = {
            "qT": np.ascontiguousarray(q[b].T.astype(bf)),
            "kT": np.ascontiguousarray(k[b].T.astype(bf)),
            "vT": np.ascontiguousarray(v[b].T.astype(bf)),
            "wq": packw(Wq[:, g * G:(g + 1) * G]),
            "wk": packw(Wk[:, g * G:(g + 1) * G]),
            "wv": packw(Wv[:, g * G:(g + 1) * G]),
            "wo": np.ascontiguousarray(
                wo_sl.reshape(2, 128, D).transpose(1, 0, 2).reshape(128, 2 * D)
                .astype(bf)),
        }
        if mode == "masked":
            im["maskT"] = np.ascontiguousarray(
                mask.T.astype(ml_dtypes.bfloat16))
        in_maps.append(im)

    res = run_bass_kernel_spmd(nc, in_maps, list(range(8)), trace=_trace,
                               **(_trace_kwargs or {}))
    outs = [np.asarray(res.results[c]["out"]).astype(np.float32)
            for c in range(8)]
    full = np.stack([outs[4 * b] + outs[4 * b + 1] + outs[4 * b + 2]
                     + outs[4 * b + 3] for b in range(B)])
    if _trace:
        return full, res
    return full


# revision 6
# speedup vs baseline: 1.1365x; 1.0039x over previous
"""Multi-head causal attention (B=2, T=2048, D=1024, H=16, dk=dv=64) on 8 NeuronCores.

Sharding: data parallel over batch (2) x tensor parallel over heads (4 groups of 4).
Core c handles batch c//4, heads [4*(c%4), 4*(c%4)+4). Each core computes the
partial output sum over its 4 heads; host adds the 4 partials per batch.

Per-core pipeline (the O-matmul uses et as STATIONARY and V_aug [128 keys, 65]
as MOVING -> 65 moving-cols per (key-tile x query-tile) instead of 512):
  QT/KT [256, T] = W.T @ xT        (bf16 matmuls, PSUM k-accumulation)
  V_aug [T, 65/head] (65th col = ones) via direct [T-part, G] projection
  per head, per tq-slice (512), per tk-tile-pair (2x128), causal-skipped:
    ST block [tk 128, tq 512] = KT_h-slice @ QT_h      (K=64)
    ET = exp(0.125 * ST)  (ACT, PSUM->SBUF bf16, batched over 2 blocks)
    diag blocks: ET *= 0/1 mask (DVE)
  per q-tile (128): O_aug [128 q, 65] += et-slice.T @ V_aug_h  (M=65 matmuls,
    et stationary); O-block lags the S-block by one slice to hide exp latency
    normalize: stg = O[:, 0:64] * recip(O[:, 64]) (per-partition scalar)
  per pair, q-tile: PE-transpose stg [q, dv2] -> otu [dv2, q]
  out [T, 1024] = otu.T @ Wo  (partial over this core's 4 heads, bf16 out)
"""
import sys

sys.path.insert(0, "/opt/trn_rl_repo")

import functools
import os
import ml_dtypes
import numpy as np

import concourse.bass as bass
import concourse.tile as tile
from concourse import mybir
from concourse.bass_utils import run_bass_kernel_spmd
from concourse.masks import make_identity

B, T, D = 2, 2048, 1024
H, DK = 16, 64            # total heads
HG = 4                    # heads per core
G = HG * DK               # 256: per-core column group width
NKT = D // 128            # 8 k-tiles of the model dim
NT = T // 128             # 16 tk tiles
NS = 4                    # tq slices
TQ = T // NS              # 512
NQT = TQ // 128           # 4 q-tiles per slice
F32 = mybir.dt.float32
BF16 = mybir.dt.bfloat16
IN_DT = BF16  # dtype for x / Wq / Wk / Wv (projection operands)
N_WARMUP = int(os.environ.get("KWARM", "400"))      # cap on filler warmups
USE_DIV = bool(int(os.environ.get("KDIV", "0")))
PAIR_BUDGET = float(os.environ.get("KPB", "250"))   # filler ns per S-pair
PE_CY = 1.0 / 2.4                                   # ns per PE cycle (ramped)


def split_multi_waits(nc, max_waits=1):
    """This walrus build has tiny per-instruction sync-wait slot limits (1 for
    matmul LW, ~2 for CTRL). Move excess waits onto preceding same-engine
    NOPs - identical semantics since each engine executes serially."""
    for func in nc.m.functions:
        for bb in func.blocks:
            out = []
            for inst in list(bb.instructions):
                si = inst.sync_info
                waits = list(si.on_wait) if (si and si.on_wait) else []
                if len(waits) > max_waits:
                    extra, keep = waits[:-max_waits], waits[-max_waits:]
                    for j, w in enumerate(extra):
                        nop = mybir.InstNoOp(name=f"{inst.name}-ws{j}")
                        nop.engine = inst.engine
                        nop.sync_info = mybir.SyncInfo(on_wait=[w], on_update=[])
                        out.append(nop)
                    inst.sync_info = mybir.SyncInfo(
                        on_wait=keep, on_update=list(si.on_update or []))
                out.append(inst)
            bb.instructions = out


def _n_alive(s, mode):
    """Number of tk tiles needed for tq slice s."""
    return NT if mode != "causal" else (TQ // 128) * (s + 1)


def _n_alive_qt(s, qt, mode):
    """Number of tk tiles needed for q-tile qt of slice s (128-granular)."""
    return NT if mode != "causal" else (TQ // 128) * s + qt + 1


@functools.lru_cache(maxsize=4)
def build_program(mode, _env=None):
    assert mode in ("causal", "dense", "masked")
    nc = bass.Bass()
    qT = nc.dram_tensor("qT", [D, T], IN_DT, kind="ExternalInput")
    kTt = nc.dram_tensor("kT", [D, T], IN_DT, kind="ExternalInput")
    vT = nc.dram_tensor("vT", [D, T], IN_DT, kind="ExternalInput")
    # weights pre-packed on host into SBUF layout: [128, NKT*G] with
    # partition p holding wq[kk*128+p, :] at cols [kk*G, (kk+1)*G)
    wq = nc.dram_tensor("wq", [128, NKT * G], IN_DT, kind="ExternalInput")
    wk = nc.dram_tensor("wk", [128, NKT * G], IN_DT, kind="ExternalInput")
    wv = nc.dram_tensor("wv", [128, NKT * G], IN_DT, kind="ExternalInput")
    wo = nc.dram_tensor("wo", [128, 2 * D], BF16, kind="ExternalInput")
    out = nc.dram_tensor("out", [T, D], BF16, kind="ExternalOutput")
    DBG = bool(int(os.environ.get("KDBG", "0")))
    dbg = {}
    if DBG:
        for nm, shape, dt_ in [("dqt0", [128, T], BF16),
                               ("dkt0", [128, T], BF16),
                               ("dva0", [128, NT * (DK + 1)], BF16),
                               ("dstg00", [128, NQT * 128], BF16),
                               ("dotu00", [128, TQ], BF16),
                               ("det", [128, 2 * TQ], BF16)]:
            dbg[nm] = nc.dram_tensor(nm, shape, dt_, kind="ExternalOutput")
    maskd = None
    if mode == "masked":
        maskd = nc.dram_tensor("maskT", [T, T], BF16, kind="ExternalInput")

    with tile.TileContext(nc) as tc:
        with (
            tc.tile_pool(name="sing", bufs=1) as sing,
            tc.tile_pool(name="xbig", bufs=1) as xbig,
            tc.tile_pool(name="etp", bufs=18) as etp,
            tc.tile_pool(name="ost", bufs=4) as ostp,
            tc.tile_pool(name="rcpp", bufs=4) as rcpp,
        ):
            # ---------------- constants ----------------
            warm = sing.tile([128, 128], BF16)
            nc.vector.memset(warm[:], 0.0)
            wq_s = sing.tile([128, NKT * G], IN_DT)
            wk_s = sing.tile([128, NKT * G], IN_DT)
            wv_s = sing.tile([128, NKT * G], IN_DT)
            wo_s = sing.tile([128, 2 * D], BF16)
            ones_sb = sing.tile([128, NT], BF16)
            nc.vector.memset(ones_sb[:], 1.0)
            ident = sing.tile([128, 128], BF16)
            make_identity(nc, ident[:])
            qt = [sing.tile([128, T], BF16, name=f"qt{p}") for p in range(2)]
            kt = [sing.tile([128, T], BF16, name=f"kt{p}") for p in range(2)]
            va = [sing.tile([128, NT * (DK + 1)], BF16, name=f"va{h}")
                  for h in range(HG)]
            # aug-last: ones col at 64 of each 65-wide group (rowsum row)
            for h in range(HG):
                nc.vector.tensor_copy(va[h][:, DK::DK + 1], ones_sb[:])
            # normalized per-(pair, slice) outputs [q, dv-pair], q-tile major
            stg = [[sing.tile([128, NQT * 128], BF16, name=f"stg{p}_{s}")
                    for s in range(NS)] for p in range(2)]

            # ------- fused projections + attention (single scheduling region)
            otrs_cm = tc.tile_pool(name="otrs", bufs=1)
            otrs = otrs_cm.__enter__()
            otu = [[otrs.tile([128, TQ], BF16, name=f"otu{p}_{s}")
                    for s in range(NS)] for p in range(2)]
            with nc.named_scope("attn"), \
                 tc.tile_pool(name="sps", bufs=2, space="PSUM") as sps, \
                 tc.tile_pool(name="mtp", bufs=4) as mtp:
                etl = {}  # (h, s) -> list of et tiles

                def emit_S(h, s):
                    """S-matmuls + exp + diag masks for head h, slice s."""
                    p, half = h // 2, h % 2
                    po = half * DK
                    na = _n_alive(s, mode)
                    etl[(h, s)] = []
                    for tp2 in range(na // 2):
                        s_ps = sps.tile([128, 2 * TQ], F32,
                                        name=f"s{h}_{s}_{tp2}", tag="s")
                        # diag blocks: cols f < 128*d are masked for every
                        # partition -> skip in S/exp/mask/O. t=0 is always
                        # full width, so PSUM accumulation start covers all.
                        c0s, ds = [], []
                        for j in range(2):
                            t = 2 * tp2 + j
                            if mode == "causal" and t >= (TQ // 128) * s:
                                d = (128 * t - TQ * s) // 128
                                ds.append(d); c0s.append(128 * d)
                            else:
                                ds.append(None); c0s.append(0)
                        for j in range(2):
                            t = 2 * tp2 + j
                            c0 = c0s[j]
                            nc.tensor.matmul(
                                s_ps[:, j * TQ + c0:(j + 1) * TQ],
                                kt[p][po:po + DK, t * 128:(t + 1) * 128],
                                qt[p][po:po + DK, s * TQ + c0:(s + 1) * TQ],
                                start=True, stop=True)
                        et = etp.tile([128, 2 * TQ], BF16,
                                      name=f"et{h}_{s}_{tp2}", tag="et")
                        # One exp instruction costs ~350 extra cycles;
                        # splitting to skip dead columns only pays off when
                        # the skip is > 128 cols. For small c0 exp the dead
                        # region too (harmless: the O-matmul never reads
                        # it), starting at min(c0s).
                        if max(c0s) <= 128:
                            cm = min(c0s)
                            nc.scalar.activation(
                                et[:, cm:2 * TQ], s_ps[:, cm:2 * TQ],
                                mybir.ActivationFunctionType.Exp,
                                scale=1.0 / np.sqrt(DK))
                        else:
                            for j in range(2):
                                c0 = c0s[j]
                                nc.scalar.activation(
                                    et[:, j * TQ + c0:(j + 1) * TQ],
                                    s_ps[:, j * TQ + c0:(j + 1) * TQ],
                                    mybir.ActivationFunctionType.Exp,
                                    scale=1.0 / np.sqrt(DK))
                        for j in range(2):
                            t = 2 * tp2 + j
                            if ds[j] is not None:
                                c0 = c0s[j]
                                # causal diag block: zero cols < row, on the
                                # otherwise-idle Pool engine (keeps DVE off
                                # the exp->mask->O critical chain)
                                nc.gpsimd.affine_select(
                                    out=et[:, j * TQ + c0:j * TQ + c0 + 128],
                                    in_=et[:, j * TQ + c0:j * TQ + c0 + 128],
                                    compare_op=mybir.AluOpType.is_ge,
                                    fill=0.0, base=0, channel_multiplier=-1,
                                    pattern=[[1, 128]])
                            elif mode == "masked":
                                mt = mtp.tile([128, TQ], BF16,
                                              name=f"mt{h}{s}{t}", tag="mt")
                                nc.sync.dma_start(
                                    out=mt,
                                    in_=maskd[t * 128:(t + 1) * 128,
                                              s * TQ:(s + 1) * TQ])
                                nc.vector.tensor_mul(
                                    et[:, j * TQ:(j + 1) * TQ],
                                    et[:, j * TQ:(j + 1) * TQ], mt[:])
                        if DBG and h == 0 and s == 0 and tp2 == 0:
                            nc.sync.dma_start(out=dbg["det"][:], in_=et[:])
                        etl[(h, s)].append(et)
                        padv((2 * TQ - c0s[0] - c0s[1]) * PE_CY)
                        drain(PAIR_BUDGET)

                def emit_O(h, s, opool, wide=False, norm_act=False):
                    """Flipped O-matmuls (et stationary, V_aug moving) +
                    per-q-tile normalization into stg."""
                    p, half = h // 2, h % 2
                    po = half * DK
                    force(("v", v_need(s)))
                    ets = etl.pop((h, s))
                    if wide:
                        o_ps = opool.tile([128, TQ], F32,
                                          name=f"o{h}_{s}", tag="op")
                    else:
                        o_ps = opool.tile([128, NQT * (DK + 1)], F32,
                                          name=f"o{h}_{s}", tag="o")
                    rcp = rcpp.tile([128, NQT], F32, name=f"r{h}_{s}",
                                    tag="rcp")
                    for qtl in range(NQT):
                        naq = _n_alive_qt(s, qtl, mode)
                        reg = o_ps[:, qtl * (DK + 1):(qtl + 1) * (DK + 1)]
                        for t in range(naq):
                            et = ets[t // 2]
                            j = t % 2
                            nc.tensor.matmul(
                                reg,
                                et[:, j * TQ + qtl * 128:
                                   j * TQ + (qtl + 1) * 128],
                                va[h][:, t * (DK + 1):(t + 1) * (DK + 1)],
                                start=(t == 0), stop=(t == naq - 1))
                        padv(naq * (DK + 1) * PE_CY)
                        drain(250.0)
                        if USE_DIV:
                            nc.vector.tensor_scalar(
                                out=stg[p][s][:, qtl * 128 + po:
                                              qtl * 128 + po + DK],
                                in0=reg[:, 0:DK],
                                scalar1=reg[:, DK:DK + 1],
                                scalar2=None,
                                op0=mybir.AluOpType.divide)
                    if not USE_DIV:
                        # one batched reciprocal over the 4 rowsums, then one
                        # per-partition-scalar multiply per q-tile (on ACT
                        # for the endgame slices, where the exp stream has
                        # drained and DVE is the congested engine)
                        nc.vector.reciprocal(
                            rcp[:], o_ps[:, DK:NQT * (DK + 1):DK + 1])
                        for qtl in range(NQT):
                            if norm_act:
                                nc.scalar.activation(
                                    stg[p][s][:, qtl * 128 + po:
                                              qtl * 128 + po + DK],
                                    o_ps[:, qtl * (DK + 1):
                                         qtl * (DK + 1) + DK],
                                    mybir.ActivationFunctionType.Copy,
                                    scale=rcp[:, qtl:qtl + 1])
                            else:
                                nc.vector.tensor_scalar_mul(
                                    stg[p][s][:, qtl * 128 + po:
                                              qtl * 128 + po + DK],
                                    o_ps[:, qtl * (DK + 1):
                                         qtl * (DK + 1) + DK],
                                    rcp[:, qtl:qtl + 1])
                    if DBG and h == 1 and s == 0:
                        nc.sync.dma_start(out=dbg["dstg00"][:],
                                          in_=stg[0][0][:])

                def emit_trans(s, m, fps, act=False):
                    """Pair transposes for q-tile m of slice s into an fps
                    slot (same byte size as the outproj psum -> shared tag),
                    then copies into otu (ACT for the final tail, where
                    the exp stream has drained and ACT idles)."""
                    tpt = fps.tile([128, D], BF16, name=f"tp{s}_{m}",
                                   tag="op")
                    for p in range(2):
                        nc.tensor.transpose(
                            tpt[:, p * 128:(p + 1) * 128],
                            stg[p][s][:, m * 128:(m + 1) * 128],
                            ident[:])
                    for p in range(2):
                        eng = nc.scalar if (act and p == 1) else nc.vector
                        if eng is nc.scalar:
                            nc.scalar.copy(
                                otu[p][s][:, m * 128:(m + 1) * 128],
                                tpt[:, p * 128:(p + 1) * 128])
                        else:
                            nc.vector.tensor_copy(
                                otu[p][s][:, m * 128:(m + 1) * 128],
                                tpt[:, p * 128:(p + 1) * 128])
                    padv(107.0)

                def emit_op(s, m, fps, act=False):
                    """Output projection for q-tile m of slice s. The final
                    tail's staging copies go to the otherwise-idle ACT."""
                    r0 = (s * NQT + m) * 128
                    o_sb = ostp.tile([128, D], BF16, name=f"os{s}_{m}",
                                     tag="os")
                    for n in range(2):
                        o_ps2 = fps.tile([128, TQ], F32,
                                         name=f"op{s}_{m}_{n}", tag="op")
                        for p in range(2):
                            nc.tensor.matmul(
                                o_ps2[:],
                                otu[p][s][:, m * 128:(m + 1) * 128],
                                wo_s[:, p * D + n * TQ:
                                     p * D + (n + 1) * TQ],
                                start=(p == 0), stop=(p == 1))
                        if act and n == 1:
                            nc.scalar.copy(
                                o_sb[:, n * TQ:(n + 1) * TQ], o_ps2[:])
                        else:
                            nc.vector.tensor_copy(
                                o_sb[:, n * TQ:(n + 1) * TQ], o_ps2[:])
                        nc.sync.dma_start(
                            out=out[r0:r0 + 128, n * TQ:(n + 1) * TQ],
                            in_=o_sb[:, n * TQ:(n + 1) * TQ])
                    padv(854.0)

                def enqueue_tail(s, fps, act=False):
                    """Transpose/outproj ladder enqueued as paced fillers
                    (drained between later S-pairs); transposes run one
                    q-tile ahead of the outproj so the otu copies are off
                    the PE wait chain."""
                    q_push(lambda: emit_trans(s, 0, fps, act), 107.0, 0.0)
                    for m in range(NQT):
                        if m + 1 < NQT:
                            q_push(lambda m=m: emit_trans(s, m + 1, fps,
                                                          act),
                                   107.0, 0.0)
                        q_push(lambda m=m: emit_op(s, m, fps, act),
                               854.0, 0.0)

                # ---- DMA emission + arrival estimates (DMA engines are a
                # single serial resource; the Tile scheduler preserves
                # per-engine emission order, so all overlap is hand-paced
                # with a filler queue driven by these estimates).
                DMA_LAT, SEM_LAT, FULL_T, HALF_T = 1850.0, 900.0, 1570.0, 800.0
                dma_t = [0.0]

                def dma_in(dst, src, ns):
                    nc.sync.dma_start(out=dst, in_=src)
                    dma_t[0] += ns
                    return DMA_LAT + dma_t[0] + SEM_LAT

                arr_wq = dma_in(wq_s[:], wq[:], FULL_T)
                arr_wk = dma_in(wk_s[:], wk[:], FULL_T)
                # q/k/v loaded in column-half waves interleaved so useful
                # work lands as early as possible: S(·,0/1) + all n<=1
                # projection groups need only columns 0:1024 of q/k, and
                # V tiles t<8 need only columns 0:1024 of v.
                qTr = [xbig.tile([128, T], IN_DT, name=f"qTr{kk}",
                                 tag=f"xq{kk}") for kk in range(NKT)]
                kTr = [xbig.tile([128, T], IN_DT, name=f"kTr{kk}",
                                 tag=f"xk{kk}") for kk in range(NKT)]
                vTr = [xbig.tile([128, T], IN_DT, name=f"vTr{kk}",
                                 tag=f"xv{kk}") for kk in range(NKT)]

                def half_wave(tiles, src, w):
                    lo, hi = w * (T // 2), (w + 1) * (T // 2)
                    return [dma_in(tiles[kk][:, lo:hi],
                                   src[kk * 128:(kk + 1) * 128, lo:hi],
                                   HALF_T) for kk in range(NKT)]

                arr_qh, arr_kh, arr_vw = [], [], []
                arr_qh.append(half_wave(qTr, qT, 0))
                arr_kh.append(half_wave(kTr, kTt, 0))
                arr_wv = dma_in(wv_s[:], wv[:], FULL_T)
                arr_vw.append(half_wave(vTr, vT, 0)[-1])
                arr_qh.append(half_wave(qTr, qT, 1))
                arr_kh.append(half_wave(kTr, kTt, 1))
                arr_vw.append(half_wave(vTr, vT, 1)[-1])
                dma_in(wo_s[:], wo[:], FULL_T)

                # ---- filler queue: (emit_fn, pe_ns, ready_ns, marker)
                import collections as _c
                queue = _c.deque()
                done = set()
                est_pe = [1500.0]
                warm_used = [0]
                wps = sps.tile([128, 2 * TQ], F32, name="wm", tag="s")

                def padv(ns):
                    est_pe[0] += ns

                def warm_one():
                    nc.tensor.matmul(wps[:, 0:128], warm[:], warm[:],
                                     start=True, stop=True)
                    warm_used[0] += 1
                    padv(55.0)

                def q_push(fn, pe_ns, ready, marker=None):
                    queue.append((fn, pe_ns, ready, marker))

                def drain(budget):
                    while budget > 0 and queue:
                        fn, pe_ns, ready, mk = queue[0]
                        if ready > est_pe[0] + 150:
                            break
                        queue.popleft()
                        fn()
                        est_pe[0] = max(est_pe[0], ready) + pe_ns
                        budget -= pe_ns
                        if mk:
                            done.add(mk)

                def force(marker):
                    while marker not in done and queue:
                        fn, pe_ns, ready, mk = queue[0]
                        if ready > est_pe[0] + 150 and \
                                warm_used[0] < N_WARMUP:
                            warm_one()
                            continue
                        queue.popleft()
                        fn()
                        est_pe[0] = max(est_pe[0], ready) + pe_ns
                        if mk:
                            done.add(mk)

                def drain_all():
                    while queue:
                        fn, pe_ns, ready, mk = queue.popleft()
                        fn()
                        est_pe[0] = max(est_pe[0], ready) + pe_ns
                        if mk:
                            done.add(mk)

                PJ = [None, "qk"]  # current proj psum pool + tag

                def proj_fillers(xr, w_s, dst, m, nA, nB, readys, mk):
                    tiles = {}

                    def mkf(n, kk):
                        def f():
                            if kk == 0:
                                tiles[n] = PJ[0].tile(
                                    [128, TQ], F32, name=f"pj{mk}{n}",
                                    tag=PJ[1])
                            nc.tensor.matmul(
                                tiles[n][:],
                                w_s[:, kk * G + m * 128:
                                    kk * G + (m + 1) * 128],
                                xr[kk][:, n * TQ:(n + 1) * TQ],
                                start=(kk == 0), stop=(kk == NKT - 1))
                            if kk == NKT - 1:
                                nc.vector.tensor_copy(
                                    dst[m][:, n * TQ:(n + 1) * TQ],
                                    tiles[n][:])
                        return f
                    for kk in range(NKT):
                        for n in dict.fromkeys((nA, nB)):
                            q_push(mkf(n, kk), 216.0, readys(n, kk),
                                   marker=(mk, n) if kk == NKT - 1 else None)

                def v_fillers(ts):
                    cur = {}

                    def mkf(t, kk):
                        def f():
                            if kk == 0:
                                cur[t] = qkps.tile([128, TQ], F32,
                                                   name=f"v{t}", tag="qk")
                            nc.tensor.matmul(
                                cur[t][:, 0:G],
                                vTr[kk][:, t * 128:(t + 1) * 128],
                                wv_s[:, kk * G:(kk + 1) * G],
                                start=(kk == 0), stop=(kk == NKT - 1))
                            if kk == NKT - 1:
                                for h in range(HG):
                                    nc.vector.tensor_copy(
                                        va[h][:, t * (DK + 1):
                                              t * (DK + 1) + DK],
                                        cur[t][:, h * DK:(h + 1) * DK])
                        return f
                    for t in ts:
                        rdy = max(arr_wv, arr_vw[t // (NT // 2)])
                        for kk in range(NKT):
                            q_push(mkf(t, kk), 110.0, rdy,
                                   marker=("v", t) if kk == NKT - 1 else None)

                qkps_cm = tc.tile_pool(name="qkps", bufs=2, space="PSUM")
                qkps = qkps_cm.__enter__()
                PJ[0] = qkps
                oap_cm = tc.tile_pool(name="oap", bufs=2, space="PSUM")
                oap = oap_cm.__enter__()
                rq = lambda n, kk: max(arr_wq, arr_qh[n // 2][kk])
                rk = lambda n, kk: max(arr_wk, arr_kh[n // 2][kk])
                # half-0-dependent groups first (m0 and m1 n<=1), then V
                # t<8, then the half-1 groups; m1 n=3,2 defer to phase B.
                proj_fillers(qTr, wq_s, qt, 0, 0, 1, rq, "q0")
                proj_fillers(kTr, wk_s, kt, 0, 0, 1, rk, "k0")
                proj_fillers(qTr, wq_s, qt, 1, 0, 1, rq, "q1")
                proj_fillers(kTr, wk_s, kt, 1, 0, 1, rk, "k1")
                v_fillers(range(NT // 2))
                proj_fillers(qTr, wq_s, qt, 0, 2, 3, rq, "q0")
                proj_fillers(kTr, wk_s, kt, 0, 2, 3, rk, "k0")
                v_fillers(range(NT // 2, NT))

                def s_need(s):
                    return s if mode == "causal" else NS - 1

                def v_need(s):
                    return _n_alive(s, mode) - 1

                # phase A: heads 0+1 interleaved, ascending slices (slice s
                # needs only the n<=s projection groups; the filler queue
                # supplies m0/m1 projections and the V projection, paced
                # against the DMA wave arrivals)
                for s in range(NS):
                    if s == NS - 1:
                        # m1 n=3,2 projections: filler for the big final
                        # slice of this phase (PE-idle while ACT drains
                        # both heads' exps); must complete before the qkps
                        # pool closes (drain_all below)
                        proj_fillers(qTr, wq_s, qt, 1, 3, 2, rq, "q1")
                        proj_fillers(kTr, wk_s, kt, 1, 3, 2, rk, "k1")
                    force(("k0", s_need(s)))
                    emit_S(0, s)
                    if s > 0:
                        emit_O(1, s - 1, oap)
                    emit_S(1, s)
                    emit_O(0, s, oap)
                emit_O(1, NS - 1, oap)
                drain_all()
                oap_cm.__exit__(None, None, None)
                qkps_cm.__exit__(None, None, None)
                # heads 2+3 merged, slices largest-first, with the
                # transpose/outproj tail enqueued as the PE filler: the
                # phase stays PE-bound (S+O+tail vs two heads' exps),
                # absorbing per-slice exp stalls, and the final serial
                # cascade is the smallest slice. O-psums, transposes and
                # outproj psums share one 4-slot rotating pool so WAR
                # waits sit 4 requests back and copy latencies hide.
                morder = [1, NS - 1, NS - 2, 0] if NS == 4 \
                    else list(range(NS))
                with tc.tile_pool(name="fps", bufs=4, space="PSUM") as fps:
                    PJ[0], PJ[1] = fps, "op"
                    last = len(morder) - 1
                    for i, s in enumerate(morder):
                        force(("k1", s if mode == "causal" else 2))
                        emit_S(2, s)
                        if i > 0:
                            emit_O(3, morder[i - 1], fps, wide=True)
                        emit_S(3, s)
                        emit_O(2, s, fps, wide=True, norm_act=(i == last))
                        if i > 0:
                            enqueue_tail(morder[i - 1], fps,
                                         act=(i == last))
                    # let queued tail pieces cover the last exp's latency
                    # before the final O-block
                    drain(1500.0)
                    emit_O(3, morder[-1], fps, wide=True, norm_act=True)
                    enqueue_tail(morder[-1], fps, act=True)
                    drain_all()

            if DBG:
                nc.sync.dma_start(out=dbg["dqt0"][:], in_=qt[0][:])
                nc.sync.dma_start(out=dbg["dkt0"][:], in_=kt[0][:])
                nc.sync.dma_start(out=dbg["dva0"][:], in_=va[0][:])
                nc.sync.dma_start(out=dbg["dotu00"][:], in_=otu[0][0][:])

            otrs_cm.__exit__(None, None, None)

    split_multi_waits(nc)
    return nc


def _detect_mode(mask):
    if mask.all():
        return "dense"
    if np.array_equal(mask, np.tril(np.ones((T, T), dtype=bool))):
        return "causal"
    return "masked"


def kernel(q, k, v, mask, Wq, Wk, Wv, Wo, _trace=False, _trace_kwargs=None):
    q, k, v = np.asarray(q), np.asarray(k), np.asarray(v)
    Wq, Wk, Wv, Wo = (np.asarray(Wq), np.asarray(Wk),
                      np.asarray(Wv), np.asarray(Wo))
    mask = np.asarray(mask)
    mode = _detect_mode(mask)
    nc = build_program(mode)

    in_maps = []
    for c in range(8):
        b, g = c // 4, c % 4
        bf = ml_dtypes.bfloat16

        def packw(w):  # [D, G] -> [128, NKT*G] SBUF layout
            return np.ascontiguousarray(
                w.reshape(NKT, 128, G).transpose(1, 0, 2).reshape(128, NKT * G)
                .astype(bf))

        wo_sl = Wo[g * G:(g + 1) * G, :]
        im = {
            "qT": np.ascontiguousarray(q[b].T.astype(bf)),
            "kT": np.ascontiguousarray(k[b].T.astype(bf)),
            "vT": np.ascontiguousarray(v[b].T.astype(bf)),
            "wq": packw(Wq[:, g * G:(g + 1) * G]),
            "wk": packw(Wk[:, g * G:(g + 1) * G]),
            "wv": packw(Wv[:, g * G:(g + 1) * G]),
            "wo": np.ascontiguousarray(
                wo_sl.reshape(2, 128, D).transpose(1, 0, 2).reshape(128, 2 * D)
                .astype(bf)),
        }
        if mode == "masked":
            im["maskT"] = np.ascontiguousarray(
                mask.T.astype(ml_dtypes.bfloat16))
        in_maps.append(im)

    res = run_bass_kernel_spmd(nc, in_maps, list(range(8)), trace=_trace,
                               **(_trace_kwargs or {}))
    outs = [np.asarray(res.results[c]["out"]).astype(np.float32)
            for c in range(8)]
    full = np.stack([outs[4 * b] + outs[4 * b + 1] + outs[4 * b + 2]
                     + outs[4 * b + 3] for b in range(B)])
    if _trace:
        return full, res
    return full


# revision 7
# speedup vs baseline: 1.1371x; 1.0005x over previous
"""Multi-head causal attention (B=2, T=2048, D=1024, H=16, dk=dv=64) on 8 NeuronCores.

Sharding: data parallel over batch (2) x tensor parallel over heads (4 groups of 4).
Core c handles batch c//4, heads [4*(c%4), 4*(c%4)+4). Each core computes the
partial output sum over its 4 heads; host adds the 4 partials per batch.

Per-core pipeline (the O-matmul uses et as STATIONARY and V_aug [128 keys, 65]
as MOVING -> 65 moving-cols per (key-tile x query-tile) instead of 512):
  QT/KT [256, T] = W.T @ xT        (bf16 matmuls, PSUM k-accumulation)
  V_aug [T, 65/head] (65th col = ones) via direct [T-part, G] projection
  per head, per tq-slice (512), per tk-tile-pair (2x128), causal-skipped:
    ST block [tk 128, tq 512] = KT_h-slice @ QT_h      (K=64)
    ET = exp(0.125 * ST)  (ACT, PSUM->SBUF bf16, batched over 2 blocks)
    diag blocks: ET *= 0/1 mask (DVE)
  per q-tile (128): O_aug [128 q, 65] += et-slice.T @ V_aug_h  (M=65 matmuls,
    et stationary); O-block lags the S-block by one slice to hide exp latency
    normalize: stg = O[:, 0:64] * recip(O[:, 64]) (per-partition scalar)
  per pair, q-tile: PE-transpose stg [q, dv2] -> otu [dv2, q]
  out [T, 1024] = otu.T @ Wo  (partial over this core's 4 heads, bf16 out)
"""
import sys

sys.path.insert(0, "/opt/trn_rl_repo")

import functools
import os
import ml_dtypes
import numpy as np

import concourse.bass as bass
import concourse.tile as tile
from concourse import mybir
from concourse.bass_utils import run_bass_kernel_spmd
from concourse.masks import make_identity

B, T, D = 2, 2048, 1024
H, DK = 16, 64            # total heads
HG = 4                    # heads per core
G = HG * DK               # 256: per-core column group width
NKT = D // 128            # 8 k-tiles of the model dim
NT = T // 128             # 16 tk tiles
NS = 4                    # tq slices
TQ = T // NS              # 512
NQT = TQ // 128           # 4 q-tiles per slice
F32 = mybir.dt.float32
BF16 = mybir.dt.bfloat16
IN_DT = BF16  # dtype for x / Wq / Wk / Wv (projection operands)
N_WARMUP = int(os.environ.get("KWARM", "400"))      # cap on filler warmups
USE_DIV = bool(int(os.environ.get("KDIV", "0")))
PAIR_BUDGET = float(os.environ.get("KPB", "250"))   # filler ns per S-pair
PE_CY = 1.0 / 2.4                                   # ns per PE cycle (ramped)


def split_multi_waits(nc, max_waits=1):
    """This walrus build has tiny per-instruction sync-wait slot limits (1 for
    matmul LW, ~2 for CTRL). Move excess waits onto preceding same-engine
    NOPs - identical semantics since each engine executes serially."""
    for func in nc.m.functions:
        for bb in func.blocks:
            out = []
            for inst in list(bb.instructions):
                si = inst.sync_info
                waits = list(si.on_wait) if (si and si.on_wait) else []
                if len(waits) > max_waits:
                    extra, keep = waits[:-max_waits], waits[-max_waits:]
                    for j, w in enumerate(extra):
                        nop = mybir.InstNoOp(name=f"{inst.name}-ws{j}")
                        nop.engine = inst.engine
                        nop.sync_info = mybir.SyncInfo(on_wait=[w], on_update=[])
                        out.append(nop)
                    inst.sync_info = mybir.SyncInfo(
                        on_wait=keep, on_update=list(si.on_update or []))
                out.append(inst)
            bb.instructions = out


def _n_alive(s, mode):
    """Number of tk tiles needed for tq slice s."""
    return NT if mode != "causal" else (TQ // 128) * (s + 1)


def _n_alive_qt(s, qt, mode):
    """Number of tk tiles needed for q-tile qt of slice s (128-granular)."""
    return NT if mode != "causal" else (TQ // 128) * s + qt + 1


@functools.lru_cache(maxsize=4)
def build_program(mode, _env=None):
    assert mode in ("causal", "dense", "masked")
    nc = bass.Bass()
    qT = nc.dram_tensor("qT", [D, T], IN_DT, kind="ExternalInput")
    kTt = nc.dram_tensor("kT", [D, T], IN_DT, kind="ExternalInput")
    vT = nc.dram_tensor("vT", [D, T], IN_DT, kind="ExternalInput")
    # weights pre-packed on host into SBUF layout: [128, NKT*G] with
    # partition p holding wq[kk*128+p, :] at cols [kk*G, (kk+1)*G)
    wq = nc.dram_tensor("wq", [128, NKT * G], IN_DT, kind="ExternalInput")
    wk = nc.dram_tensor("wk", [128, NKT * G], IN_DT, kind="ExternalInput")
    wv = nc.dram_tensor("wv", [128, NKT * G], IN_DT, kind="ExternalInput")
    wo = nc.dram_tensor("wo", [128, 2 * D], BF16, kind="ExternalInput")
    out = nc.dram_tensor("out", [T, D], BF16, kind="ExternalOutput")
    DBG = bool(int(os.environ.get("KDBG", "0")))
    dbg = {}
    if DBG:
        for nm, shape, dt_ in [("dqt0", [128, T], BF16),
                               ("dkt0", [128, T], BF16),
                               ("dva0", [128, NT * (DK + 1)], BF16),
                               ("dstg00", [128, NQT * 128], BF16),
                               ("dotu00", [128, TQ], BF16),
                               ("det", [128, 2 * TQ], BF16)]:
            dbg[nm] = nc.dram_tensor(nm, shape, dt_, kind="ExternalOutput")
    maskd = None
    if mode == "masked":
        maskd = nc.dram_tensor("maskT", [T, T], BF16, kind="ExternalInput")

    with tile.TileContext(nc) as tc:
        with (
            tc.tile_pool(name="sing", bufs=1) as sing,
            tc.tile_pool(name="xbig", bufs=1) as xbig,
            tc.tile_pool(name="etp", bufs=18) as etp,
            tc.tile_pool(name="ost", bufs=4) as ostp,
            tc.tile_pool(name="rcpp", bufs=4) as rcpp,
        ):
            # ---------------- constants ----------------
            warm = sing.tile([128, 128], BF16)
            nc.vector.memset(warm[:], 0.0)
            wq_s = sing.tile([128, NKT * G], IN_DT)
            wk_s = sing.tile([128, NKT * G], IN_DT)
            wv_s = sing.tile([128, NKT * G], IN_DT)
            wo_s = sing.tile([128, 2 * D], BF16)
            ones_sb = sing.tile([128, NT], BF16)
            nc.vector.memset(ones_sb[:], 1.0)
            ident = sing.tile([128, 128], BF16)
            make_identity(nc, ident[:])
            qt = [sing.tile([128, T], BF16, name=f"qt{p}") for p in range(2)]
            kt = [sing.tile([128, T], BF16, name=f"kt{p}") for p in range(2)]
            va = [sing.tile([128, NT * (DK + 1)], BF16, name=f"va{h}")
                  for h in range(HG)]
            # aug-last: ones col at 64 of each 65-wide group (rowsum row)
            for h in range(HG):
                nc.vector.tensor_copy(va[h][:, DK::DK + 1], ones_sb[:])
            # normalized per-(pair, slice) outputs [q, dv-pair], q-tile major
            stg = [[sing.tile([128, NQT * 128], BF16, name=f"stg{p}_{s}")
                    for s in range(NS)] for p in range(2)]

            # ------- fused projections + attention (single scheduling region)
            otrs_cm = tc.tile_pool(name="otrs", bufs=1)
            otrs = otrs_cm.__enter__()
            otu = [[otrs.tile([128, TQ], BF16, name=f"otu{p}_{s}")
                    for s in range(NS)] for p in range(2)]
            with nc.named_scope("attn"), \
                 tc.tile_pool(name="sps", bufs=2, space="PSUM") as sps, \
                 tc.tile_pool(name="mtp", bufs=4) as mtp:
                etl = {}  # (h, s) -> list of et tiles

                def emit_S(h, s):
                    """S-matmuls + exp + diag masks for head h, slice s."""
                    p, half = h // 2, h % 2
                    po = half * DK
                    na = _n_alive(s, mode)
                    etl[(h, s)] = []
                    for tp2 in range(na // 2):
                        s_ps = sps.tile([128, 2 * TQ], F32,
                                        name=f"s{h}_{s}_{tp2}", tag="s")
                        # diag blocks: cols f < 128*d are masked for every
                        # partition -> skip in S/exp/mask/O. t=0 is always
                        # full width, so PSUM accumulation start covers all.
                        c0s, ds = [], []
                        for j in range(2):
                            t = 2 * tp2 + j
                            if mode == "causal" and t >= (TQ // 128) * s:
                                d = (128 * t - TQ * s) // 128
                                ds.append(d); c0s.append(128 * d)
                            else:
                                ds.append(None); c0s.append(0)
                        for j in range(2):
                            t = 2 * tp2 + j
                            c0 = c0s[j]
                            nc.tensor.matmul(
                                s_ps[:, j * TQ + c0:(j + 1) * TQ],
                                kt[p][po:po + DK, t * 128:(t + 1) * 128],
                                qt[p][po:po + DK, s * TQ + c0:(s + 1) * TQ],
                                start=True, stop=True)
                        et = etp.tile([128, 2 * TQ], BF16,
                                      name=f"et{h}_{s}_{tp2}", tag="et")
                        # One exp instruction costs ~350 extra cycles;
                        # splitting to skip dead columns only pays off when
                        # the skip is > 128 cols. For small c0 exp the dead
                        # region too (harmless: the O-matmul never reads
                        # it), starting at min(c0s).
                        if max(c0s) <= 128:
                            cm = min(c0s)
                            nc.scalar.activation(
                                et[:, cm:2 * TQ], s_ps[:, cm:2 * TQ],
                                mybir.ActivationFunctionType.Exp,
                                scale=1.0 / np.sqrt(DK))
                        else:
                            for j in range(2):
                                c0 = c0s[j]
                                nc.scalar.activation(
                                    et[:, j * TQ + c0:(j + 1) * TQ],
                                    s_ps[:, j * TQ + c0:(j + 1) * TQ],
                                    mybir.ActivationFunctionType.Exp,
                                    scale=1.0 / np.sqrt(DK))
                        for j in range(2):
                            t = 2 * tp2 + j
                            if ds[j] is not None:
                                c0 = c0s[j]
                                # causal diag block: zero cols < row, on the
                                # otherwise-idle Pool engine (keeps DVE off
                                # the exp->mask->O critical chain)
                                nc.gpsimd.affine_select(
                                    out=et[:, j * TQ + c0:j * TQ + c0 + 128],
                                    in_=et[:, j * TQ + c0:j * TQ + c0 + 128],
                                    compare_op=mybir.AluOpType.is_ge,
                                    fill=0.0, base=0, channel_multiplier=-1,
                                    pattern=[[1, 128]])
                            elif mode == "masked":
                                mt = mtp.tile([128, TQ], BF16,
                                              name=f"mt{h}{s}{t}", tag="mt")
                                nc.sync.dma_start(
                                    out=mt,
                                    in_=maskd[t * 128:(t + 1) * 128,
                                              s * TQ:(s + 1) * TQ])
                                nc.vector.tensor_mul(
                                    et[:, j * TQ:(j + 1) * TQ],
                                    et[:, j * TQ:(j + 1) * TQ], mt[:])
                        if DBG and h == 0 and s == 0 and tp2 == 0:
                            nc.sync.dma_start(out=dbg["det"][:], in_=et[:])
                        etl[(h, s)].append(et)
                        padv((2 * TQ - c0s[0] - c0s[1]) * PE_CY)
                        drain(PAIR_BUDGET)

                def emit_O(h, s, opool, wide=False, norm_act=False):
                    """Flipped O-matmuls (et stationary, V_aug moving) +
                    per-q-tile normalization into stg."""
                    p, half = h // 2, h % 2
                    po = half * DK
                    force(("v", v_need(s)))
                    ets = etl.pop((h, s))
                    if wide:
                        o_ps = opool.tile([128, TQ], F32,
                                          name=f"o{h}_{s}", tag="op")
                    else:
                        o_ps = opool.tile([128, NQT * (DK + 1)], F32,
                                          name=f"o{h}_{s}", tag="o")
                    rcp = rcpp.tile([128, NQT], F32, name=f"r{h}_{s}",
                                    tag="rcp")
                    for qtl in range(NQT):
                        naq = _n_alive_qt(s, qtl, mode)
                        reg = o_ps[:, qtl * (DK + 1):(qtl + 1) * (DK + 1)]
                        for t in range(naq):
                            et = ets[t // 2]
                            j = t % 2
                            nc.tensor.matmul(
                                reg,
                                et[:, j * TQ + qtl * 128:
                                   j * TQ + (qtl + 1) * 128],
                                va[h][:, t * (DK + 1):(t + 1) * (DK + 1)],
                                start=(t == 0), stop=(t == naq - 1))
                        padv(naq * (DK + 1) * PE_CY)
                        drain(150.0)
                        if USE_DIV:
                            nc.vector.tensor_scalar(
                                out=stg[p][s][:, qtl * 128 + po:
                                              qtl * 128 + po + DK],
                                in0=reg[:, 0:DK],
                                scalar1=reg[:, DK:DK + 1],
                                scalar2=None,
                                op0=mybir.AluOpType.divide)
                    if not USE_DIV:
                        # one batched reciprocal over the 4 rowsums, then one
                        # per-partition-scalar multiply per q-tile (on ACT
                        # for the endgame slices, where the exp stream has
                        # drained and DVE is the congested engine)
                        nc.vector.reciprocal(
                            rcp[:], o_ps[:, DK:NQT * (DK + 1):DK + 1])
                        for qtl in range(NQT):
                            if norm_act:
                                nc.scalar.activation(
                                    stg[p][s][:, qtl * 128 + po:
                                              qtl * 128 + po + DK],
                                    o_ps[:, qtl * (DK + 1):
                                         qtl * (DK + 1) + DK],
                                    mybir.ActivationFunctionType.Copy,
                                    scale=rcp[:, qtl:qtl + 1])
                            else:
                                nc.vector.tensor_scalar_mul(
                                    stg[p][s][:, qtl * 128 + po:
                                              qtl * 128 + po + DK],
                                    o_ps[:, qtl * (DK + 1):
                                         qtl * (DK + 1) + DK],
                                    rcp[:, qtl:qtl + 1])
                    if DBG and h == 1 and s == 0:
                        nc.sync.dma_start(out=dbg["dstg00"][:],
                                          in_=stg[0][0][:])

                def emit_trans(s, m, fps, act=False):
                    """Pair transposes for q-tile m of slice s into an fps
                    slot (same byte size as the outproj psum -> shared tag),
                    then copies into otu (ACT for the final tail, where
                    the exp stream has drained and ACT idles)."""
                    tpt = fps.tile([128, D], BF16, name=f"tp{s}_{m}",
                                   tag="op")
                    for p in range(2):
                        nc.tensor.transpose(
                            tpt[:, p * 128:(p + 1) * 128],
                            stg[p][s][:, m * 128:(m + 1) * 128],
                            ident[:])
                    for p in range(2):
                        eng = nc.scalar if (act and p == 1) else nc.vector
                        if eng is nc.scalar:
                            nc.scalar.copy(
                                otu[p][s][:, m * 128:(m + 1) * 128],
                                tpt[:, p * 128:(p + 1) * 128])
                        else:
                            nc.vector.tensor_copy(
                                otu[p][s][:, m * 128:(m + 1) * 128],
                                tpt[:, p * 128:(p + 1) * 128])
                    padv(107.0)

                def emit_op(s, m, fps, act=False):
                    """Output projection for q-tile m of slice s. The final
                    tail's staging copies go to the otherwise-idle ACT."""
                    r0 = (s * NQT + m) * 128
                    o_sb = ostp.tile([128, D], BF16, name=f"os{s}_{m}",
                                     tag="os")
                    for n in range(2):
                        o_ps2 = fps.tile([128, TQ], F32,
                                         name=f"op{s}_{m}_{n}", tag="op")
                        for p in range(2):
                            nc.tensor.matmul(
                                o_ps2[:],
                                otu[p][s][:, m * 128:(m + 1) * 128],
                                wo_s[:, p * D + n * TQ:
                                     p * D + (n + 1) * TQ],
                                start=(p == 0), stop=(p == 1))
                        if act and n == 1:
                            nc.scalar.copy(
                                o_sb[:, n * TQ:(n + 1) * TQ], o_ps2[:])
                        else:
                            nc.vector.tensor_copy(
                                o_sb[:, n * TQ:(n + 1) * TQ], o_ps2[:])
                        nc.sync.dma_start(
                            out=out[r0:r0 + 128, n * TQ:(n + 1) * TQ],
                            in_=o_sb[:, n * TQ:(n + 1) * TQ])
                    padv(854.0)

                def enqueue_tail(s, fps, act=False):
                    """Transpose/outproj ladder enqueued as paced fillers
                    (drained between later S-pairs); transposes run one
                    q-tile ahead of the outproj so the otu copies are off
                    the PE wait chain."""
                    q_push(lambda: emit_trans(s, 0, fps, act), 107.0, 0.0)
                    for m in range(NQT):
                        if m + 1 < NQT:
                            q_push(lambda m=m: emit_trans(s, m + 1, fps,
                                                          act),
                                   107.0, 0.0)
                        q_push(lambda m=m: emit_op(s, m, fps, act),
                               854.0, 0.0)

                # ---- DMA emission + arrival estimates (DMA engines are a
                # single serial resource; the Tile scheduler preserves
                # per-engine emission order, so all overlap is hand-paced
                # with a filler queue driven by these estimates).
                DMA_LAT, SEM_LAT, FULL_T, HALF_T = 1850.0, 900.0, 1570.0, 800.0
                dma_t = [0.0]

                def dma_in(dst, src, ns):
                    nc.sync.dma_start(out=dst, in_=src)
                    dma_t[0] += ns
                    return DMA_LAT + dma_t[0] + SEM_LAT

                arr_wq = dma_in(wq_s[:], wq[:], FULL_T)
                arr_wk = dma_in(wk_s[:], wk[:], FULL_T)
                # q/k/v loaded in column-half waves interleaved so useful
                # work lands as early as possible: S(·,0/1) + all n<=1
                # projection groups need only columns 0:1024 of q/k, and
                # V tiles t<8 need only columns 0:1024 of v.
                qTr = [xbig.tile([128, T], IN_DT, name=f"qTr{kk}",
                                 tag=f"xq{kk}") for kk in range(NKT)]
                kTr = [xbig.tile([128, T], IN_DT, name=f"kTr{kk}",
                                 tag=f"xk{kk}") for kk in range(NKT)]
                vTr = [xbig.tile([128, T], IN_DT, name=f"vTr{kk}",
                                 tag=f"xv{kk}") for kk in range(NKT)]

                def half_wave(tiles, src, w):
                    lo, hi = w * (T // 2), (w + 1) * (T // 2)
                    return [dma_in(tiles[kk][:, lo:hi],
                                   src[kk * 128:(kk + 1) * 128, lo:hi],
                                   HALF_T) for kk in range(NKT)]

                arr_qh, arr_kh, arr_vw = [], [], []
                arr_qh.append(half_wave(qTr, qT, 0))
                arr_kh.append(half_wave(kTr, kTt, 0))
                arr_wv = dma_in(wv_s[:], wv[:], FULL_T)
                arr_vw.append(half_wave(vTr, vT, 0)[-1])
                arr_qh.append(half_wave(qTr, qT, 1))
                arr_kh.append(half_wave(kTr, kTt, 1))
                arr_vw.append(half_wave(vTr, vT, 1)[-1])
                dma_in(wo_s[:], wo[:], FULL_T)

                # ---- filler queue: (emit_fn, pe_ns, ready_ns, marker)
                import collections as _c
                queue = _c.deque()
                done = set()
                est_pe = [1500.0]
                warm_used = [0]
                wps = sps.tile([128, 2 * TQ], F32, name="wm", tag="s")

                def padv(ns):
                    est_pe[0] += ns

                def warm_one():
                    nc.tensor.matmul(wps[:, 0:128], warm[:], warm[:],
                                     start=True, stop=True)
                    warm_used[0] += 1
                    padv(55.0)

                def q_push(fn, pe_ns, ready, marker=None):
                    queue.append((fn, pe_ns, ready, marker))

                def drain(budget):
                    while budget > 0 and queue:
                        fn, pe_ns, ready, mk = queue[0]
                        if ready > est_pe[0] + 150:
                            break
                        queue.popleft()
                        fn()
                        est_pe[0] = max(est_pe[0], ready) + pe_ns
                        budget -= pe_ns
                        if mk:
                            done.add(mk)

                def force(marker):
                    while marker not in done and queue:
                        fn, pe_ns, ready, mk = queue[0]
                        if ready > est_pe[0] + 150 and \
                                warm_used[0] < N_WARMUP:
                            warm_one()
                            continue
                        queue.popleft()
                        fn()
                        est_pe[0] = max(est_pe[0], ready) + pe_ns
                        if mk:
                            done.add(mk)

                def drain_all():
                    while queue:
                        fn, pe_ns, ready, mk = queue.popleft()
                        fn()
                        est_pe[0] = max(est_pe[0], ready) + pe_ns
                        if mk:
                            done.add(mk)

                PJ = [None, "qk"]  # current proj psum pool + tag

                def proj_fillers(xr, w_s, dst, m, nA, nB, readys, mk):
                    tiles = {}

                    def mkf(n, kk):
                        def f():
                            if kk == 0:
                                tiles[n] = PJ[0].tile(
                                    [128, TQ], F32, name=f"pj{mk}{n}",
                                    tag=PJ[1])
                            nc.tensor.matmul(
                                tiles[n][:],
                                w_s[:, kk * G + m * 128:
                                    kk * G + (m + 1) * 128],
                                xr[kk][:, n * TQ:(n + 1) * TQ],
                                start=(kk == 0), stop=(kk == NKT - 1))
                            if kk == NKT - 1:
                                nc.vector.tensor_copy(
                                    dst[m][:, n * TQ:(n + 1) * TQ],
                                    tiles[n][:])
                        return f
                    for kk in range(NKT):
                        for n in dict.fromkeys((nA, nB)):
                            q_push(mkf(n, kk), 216.0, readys(n, kk),
                                   marker=(mk, n) if kk == NKT - 1 else None)

                def v_fillers(ts):
                    cur = {}

                    def mkf(t, kk):
                        def f():
                            if kk == 0:
                                cur[t] = qkps.tile([128, TQ], F32,
                                                   name=f"v{t}", tag="qk")
                            nc.tensor.matmul(
                                cur[t][:, 0:G],
                                vTr[kk][:, t * 128:(t + 1) * 128],
                                wv_s[:, kk * G:(kk + 1) * G],
                                start=(kk == 0), stop=(kk == NKT - 1))
                            if kk == NKT - 1:
                                for h in range(HG):
                                    nc.vector.tensor_copy(
                                        va[h][:, t * (DK + 1):
                                              t * (DK + 1) + DK],
                                        cur[t][:, h * DK:(h + 1) * DK])
                        return f
                    for t in ts:
                        rdy = max(arr_wv, arr_vw[t // (NT // 2)])
                        for kk in range(NKT):
                            q_push(mkf(t, kk), 110.0, rdy,
                                   marker=("v", t) if kk == NKT - 1 else None)

                qkps_cm = tc.tile_pool(name="qkps", bufs=2, space="PSUM")
                qkps = qkps_cm.__enter__()
                PJ[0] = qkps
                oap_cm = tc.tile_pool(name="oap", bufs=2, space="PSUM")
                oap = oap_cm.__enter__()
                rq = lambda n, kk: max(arr_wq, arr_qh[n // 2][kk])
                rk = lambda n, kk: max(arr_wk, arr_kh[n // 2][kk])
                # half-0-dependent groups first (m0 and m1 n<=1), then V
                # t<8, then the half-1 groups; m1 n=3,2 defer to phase B.
                proj_fillers(qTr, wq_s, qt, 0, 0, 1, rq, "q0")
                proj_fillers(kTr, wk_s, kt, 0, 0, 1, rk, "k0")
                proj_fillers(qTr, wq_s, qt, 1, 0, 1, rq, "q1")
                proj_fillers(kTr, wk_s, kt, 1, 0, 1, rk, "k1")
                v_fillers(range(NT // 2))
                proj_fillers(qTr, wq_s, qt, 0, 2, 3, rq, "q0")
                proj_fillers(kTr, wk_s, kt, 0, 2, 3, rk, "k0")
                v_fillers(range(NT // 2, NT))

                def s_need(s):
                    return s if mode == "causal" else NS - 1

                def v_need(s):
                    return _n_alive(s, mode) - 1

                # phase A: heads 0+1 interleaved, ascending slices (slice s
                # needs only the n<=s projection groups; the filler queue
                # supplies m0/m1 projections and the V projection, paced
                # against the DMA wave arrivals)
                for s in range(NS):
                    if s == NS - 1:
                        # m1 n=3,2 projections: filler for the big final
                        # slice of this phase (PE-idle while ACT drains
                        # both heads' exps); must complete before the qkps
                        # pool closes (drain_all below)
                        proj_fillers(qTr, wq_s, qt, 1, 3, 2, rq, "q1")
                        proj_fillers(kTr, wk_s, kt, 1, 3, 2, rk, "k1")
                    force(("k0", s_need(s)))
                    emit_S(0, s)
                    if s > 0:
                        emit_O(1, s - 1, oap)
                    emit_S(1, s)
                    emit_O(0, s, oap)
                emit_O(1, NS - 1, oap)
                drain_all()
                oap_cm.__exit__(None, None, None)
                qkps_cm.__exit__(None, None, None)
                # heads 2+3 merged, slices largest-first, with the
                # transpose/outproj tail enqueued as the PE filler: the
                # phase stays PE-bound (S+O+tail vs two heads' exps),
                # absorbing per-slice exp stalls, and the final serial
                # cascade is the smallest slice. O-psums, transposes and
                # outproj psums share one 4-slot rotating pool so WAR
                # waits sit 4 requests back and copy latencies hide.
                morder = [1, NS - 1, NS - 2, 0] if NS == 4 \
                    else list(range(NS))
                with tc.tile_pool(name="fps", bufs=4, space="PSUM") as fps:
                    PJ[0], PJ[1] = fps, "op"
                    last = len(morder) - 1
                    for i, s in enumerate(morder):
                        force(("k1", s if mode == "causal" else 2))
                        emit_S(2, s)
                        if i > 0:
                            emit_O(3, morder[i - 1], fps, wide=True)
                        emit_S(3, s)
                        emit_O(2, s, fps, wide=True, norm_act=(i == last))
                        if i > 0:
                            enqueue_tail(morder[i - 1], fps,
                                         act=(i == last))
                    # let queued tail pieces cover the last exp's latency
                    # before the final O-block
                    drain(1500.0)
                    emit_O(3, morder[-1], fps, wide=True, norm_act=True)
                    enqueue_tail(morder[-1], fps, act=True)
                    drain_all()

            if DBG:
                nc.sync.dma_start(out=dbg["dqt0"][:], in_=qt[0][:])
                nc.sync.dma_start(out=dbg["dkt0"][:], in_=kt[0][:])
                nc.sync.dma_start(out=dbg["dva0"][:], in_=va[0][:])
                nc.sync.dma_start(out=dbg["dotu00"][:], in_=otu[0][0][:])

            otrs_cm.__exit__(None, None, None)

    split_multi_waits(nc)
    return nc


def _detect_mode(mask):
    if mask.all():
        return "dense"
    if np.array_equal(mask, np.tril(np.ones((T, T), dtype=bool))):
        return "causal"
    return "masked"


def kernel(q, k, v, mask, Wq, Wk, Wv, Wo, _trace=False, _trace_kwargs=None):
    q, k, v = np.asarray(q), np.asarray(k), np.asarray(v)
    Wq, Wk, Wv, Wo = (np.asarray(Wq), np.asarray(Wk),
                      np.asarray(Wv), np.asarray(Wo))
    mask = np.asarray(mask)
    mode = _detect_mode(mask)
    nc = build_program(mode)

    in_maps = []
    for c in range(8):
        b, g = c // 4, c % 4
        bf = ml_dtypes.bfloat16

        def packw(w):  # [D, G] -> [128, NKT*G] SBUF layout
            return np.ascontiguousarray(
                w.reshape(NKT, 128, G).transpose(1, 0, 2).reshape(128, NKT * G)
                .astype(bf))

        wo_sl = Wo[g * G:(g + 1) * G, :]
        im = {
            "qT": np.ascontiguousarray(q[b].T.astype(bf)),
            "kT": np.ascontiguousarray(k[b].T.astype(bf)),
            "vT": np.ascontiguousarray(v[b].T.astype(bf)),
            "wq": packw(Wq[:, g * G:(g + 1) * G]),
            "wk": packw(Wk[:, g * G:(g + 1) * G]),
            "wv": packw(Wv[:, g * G:(g + 1) * G]),
            "wo": np.ascontiguousarray(
                wo_sl.reshape(2, 128, D).transpose(1, 0, 2).reshape(128, 2 * D)
                .astype(bf)),
        }
        if mode == "masked":
            im["maskT"] = np.ascontiguousarray(
                mask.T.astype(ml_dtypes.bfloat16))
        in_maps.append(im)

    res = run_bass_kernel_spmd(nc, in_maps, list(range(8)), trace=_trace,
                               **(_trace_kwargs or {}))
    outs = [np.asarray(res.results[c]["out"]).astype(np.float32)
            for c in range(8)]
    full = np.stack([outs[4 * b] + outs[4 * b + 1] + outs[4 * b + 2]
                     + outs[4 * b + 3] for b in range(B)])
    if _trace:
        return full, res
    return full


# revision 8
# speedup vs baseline: 1.1432x; 1.0054x over previous
"""Multi-head causal attention (B=2, T=2048, D=1024, H=16, dk=dv=64) on 8 NeuronCores.

Sharding: data parallel over batch (2) x tensor parallel over heads (4 groups of 4).
Core c handles batch c//4, heads [4*(c%4), 4*(c%4)+4). Each core computes the
partial output sum over its 4 heads; host adds the 4 partials per batch.

Per-core pipeline (the O-matmul uses et as STATIONARY and V_aug [128 keys, 65]
as MOVING -> 65 moving-cols per (key-tile x query-tile) instead of 512):
  QT/KT [256, T] = W.T @ xT        (bf16 matmuls, PSUM k-accumulation)
  V_aug [T, 65/head] (65th col = ones) via direct [T-part, G] projection
  per head, per tq-slice (512), per tk-tile-pair (2x128), causal-skipped:
    ST block [tk 128, tq 512] = KT_h-slice @ QT_h      (K=64)
    ET = exp(0.125 * ST)  (ACT, PSUM->SBUF bf16, batched over 2 blocks)
    diag blocks: ET *= 0/1 mask (DVE)
  per q-tile (128): O_aug [128 q, 65] += et-slice.T @ V_aug_h  (M=65 matmuls,
    et stationary); O-block lags the S-block by one slice to hide exp latency
    normalize: stg = O[:, 0:64] * recip(O[:, 64]) (per-partition scalar)
  per pair, q-tile: PE-transpose stg [q, dv2] -> otu [dv2, q]
  out [T, 1024] = otu.T @ Wo  (partial over this core's 4 heads, bf16 out)
"""
import sys

sys.path.insert(0, "/opt/trn_rl_repo")

import functools
import os
import ml_dtypes
import numpy as np

import concourse.bass as bass
import concourse.tile as tile
from concourse import mybir
from concourse.bass_utils import run_bass_kernel_spmd
from concourse.masks import make_identity

B, T, D = 2, 2048, 1024
H, DK = 16, 64            # total heads
HG = 4                    # heads per core
G = HG * DK               # 256: per-core column group width
NKT = D // 128            # 8 k-tiles of the model dim
NT = T // 128             # 16 tk tiles
NS = 4                    # tq slices
TQ = T // NS              # 512
NQT = TQ // 128           # 4 q-tiles per slice
F32 = mybir.dt.float32
BF16 = mybir.dt.bfloat16
IN_DT = BF16  # dtype for x / Wq / Wk / Wv (projection operands)
N_WARMUP = int(os.environ.get("KWARM", "400"))      # cap on filler warmups
USE_DIV = bool(int(os.environ.get("KDIV", "0")))
PAIR_BUDGET = float(os.environ.get("KPB", "250"))   # filler ns per S-pair
PE_CY = 1.0 / 2.4                                   # ns per PE cycle (ramped)


def split_multi_waits(nc, max_waits=1):
    """This walrus build has tiny per-instruction sync-wait slot limits (1 for
    matmul LW, ~2 for CTRL). Move excess waits onto preceding same-engine
    NOPs - identical semantics since each engine executes serially."""
    for func in nc.m.functions:
        for bb in func.blocks:
            out = []
            for inst in list(bb.instructions):
                si = inst.sync_info
                waits = list(si.on_wait) if (si and si.on_wait) else []
                if len(waits) > max_waits:
                    extra, keep = waits[:-max_waits], waits[-max_waits:]
                    for j, w in enumerate(extra):
                        nop = mybir.InstNoOp(name=f"{inst.name}-ws{j}")
                        nop.engine = inst.engine
                        nop.sync_info = mybir.SyncInfo(on_wait=[w], on_update=[])
                        out.append(nop)
                    inst.sync_info = mybir.SyncInfo(
                        on_wait=keep, on_update=list(si.on_update or []))
                out.append(inst)
            bb.instructions = out


def _n_alive(s, mode):
    """Number of tk tiles needed for tq slice s."""
    return NT if mode != "causal" else (TQ // 128) * (s + 1)


def _n_alive_qt(s, qt, mode):
    """Number of tk tiles needed for q-tile qt of slice s (128-granular)."""
    return NT if mode != "causal" else (TQ // 128) * s + qt + 1


@functools.lru_cache(maxsize=4)
def build_program(mode, _env=None):
    assert mode in ("causal", "dense", "masked")
    nc = bass.Bass()
    qT = nc.dram_tensor("qT", [D, T], IN_DT, kind="ExternalInput")
    kTt = nc.dram_tensor("kT", [D, T], IN_DT, kind="ExternalInput")
    vT = nc.dram_tensor("vT", [D, T], IN_DT, kind="ExternalInput")
    # weights pre-packed on host into SBUF layout: [128, NKT*G] with
    # partition p holding wq[kk*128+p, :] at cols [kk*G, (kk+1)*G)
    wq = nc.dram_tensor("wq", [128, NKT * G], IN_DT, kind="ExternalInput")
    wk = nc.dram_tensor("wk", [128, NKT * G], IN_DT, kind="ExternalInput")
    wv = nc.dram_tensor("wv", [128, NKT * G], IN_DT, kind="ExternalInput")
    wo = nc.dram_tensor("wo", [128, 2 * D], BF16, kind="ExternalInput")
    out = nc.dram_tensor("out", [T, D], BF16, kind="ExternalOutput")
    DBG = bool(int(os.environ.get("KDBG", "0")))
    dbg = {}
    if DBG:
        for nm, shape, dt_ in [("dqt0", [128, T], BF16),
                               ("dkt0", [128, T], BF16),
                               ("dva0", [128, NT * (DK + 1)], BF16),
                               ("dstg00", [128, NQT * 128], BF16),
                               ("dotu00", [128, TQ], BF16),
                               ("det", [128, 2 * TQ], BF16)]:
            dbg[nm] = nc.dram_tensor(nm, shape, dt_, kind="ExternalOutput")
    maskd = None
    if mode == "masked":
        maskd = nc.dram_tensor("maskT", [T, T], BF16, kind="ExternalInput")

    with tile.TileContext(nc) as tc:
        with (
            tc.tile_pool(name="sing", bufs=1) as sing,
            tc.tile_pool(name="xbig", bufs=1) as xbig,
            tc.tile_pool(name="etp", bufs=18) as etp,
            tc.tile_pool(name="ost", bufs=4) as ostp,
            tc.tile_pool(name="rcpp", bufs=4) as rcpp,
        ):
            # ---------------- constants ----------------
            warm = sing.tile([128, 128], BF16)
            nc.vector.memset(warm[:], 0.0)
            wq_s = sing.tile([128, NKT * G], IN_DT)
            wk_s = sing.tile([128, NKT * G], IN_DT)
            wv_s = sing.tile([128, NKT * G], IN_DT)
            wo_s = sing.tile([128, 2 * D], BF16)
            ones_sb = sing.tile([128, NT], BF16)
            nc.vector.memset(ones_sb[:], 1.0)
            ident = sing.tile([128, 128], BF16)
            make_identity(nc, ident[:])
            qt = [sing.tile([128, T], BF16, name=f"qt{p}") for p in range(2)]
            kt = [sing.tile([128, T], BF16, name=f"kt{p}") for p in range(2)]
            va = [sing.tile([128, NT * (DK + 1)], BF16, name=f"va{h}")
                  for h in range(HG)]
            # aug-last: ones col at 64 of each 65-wide group (rowsum row)
            for h in range(HG):
                nc.vector.tensor_copy(va[h][:, DK::DK + 1], ones_sb[:])
            # normalized per-(pair, slice) outputs [q, dv-pair], q-tile major
            stg = [[sing.tile([128, NQT * 128], BF16, name=f"stg{p}_{s}")
                    for s in range(NS)] for p in range(2)]

            # ------- fused projections + attention (single scheduling region)
            otrs_cm = tc.tile_pool(name="otrs", bufs=1)
            otrs = otrs_cm.__enter__()
            otu = [[otrs.tile([128, TQ], BF16, name=f"otu{p}_{s}")
                    for s in range(NS)] for p in range(2)]
            with nc.named_scope("attn"), \
                 tc.tile_pool(name="sps", bufs=2, space="PSUM") as sps, \
                 tc.tile_pool(name="mtp", bufs=4) as mtp:
                etl = {}  # (h, s) -> list of et tiles

                def emit_S(h, s):
                    """S-matmuls + exp + diag masks for head h, slice s."""
                    p, half = h // 2, h % 2
                    po = half * DK
                    na = _n_alive(s, mode)
                    etl[(h, s)] = []
                    for tp2 in range(na // 2):
                        s_ps = sps.tile([128, 2 * TQ], F32,
                                        name=f"s{h}_{s}_{tp2}", tag="s")
                        # diag blocks: cols f < 128*d are masked for every
                        # partition -> skip in S/exp/mask/O. t=0 is always
                        # full width, so PSUM accumulation start covers all.
                        c0s, ds = [], []
                        for j in range(2):
                            t = 2 * tp2 + j
                            if mode == "causal" and t >= (TQ // 128) * s:
                                d = (128 * t - TQ * s) // 128
                                ds.append(d); c0s.append(128 * d)
                            else:
                                ds.append(None); c0s.append(0)
                        for j in range(2):
                            t = 2 * tp2 + j
                            c0 = c0s[j]
                            nc.tensor.matmul(
                                s_ps[:, j * TQ + c0:(j + 1) * TQ],
                                kt[p][po:po + DK, t * 128:(t + 1) * 128],
                                qt[p][po:po + DK, s * TQ + c0:(s + 1) * TQ],
                                start=True, stop=True)
                        et = etp.tile([128, 2 * TQ], BF16,
                                      name=f"et{h}_{s}_{tp2}", tag="et")
                        # One exp instruction costs ~350 extra cycles;
                        # splitting to skip dead columns only pays off when
                        # the skip is > 128 cols. For small c0 exp the dead
                        # region too (harmless: the O-matmul never reads
                        # it), starting at min(c0s).
                        if max(c0s) <= 128:
                            cm = min(c0s)
                            nc.scalar.activation(
                                et[:, cm:2 * TQ], s_ps[:, cm:2 * TQ],
                                mybir.ActivationFunctionType.Exp,
                                scale=1.0 / np.sqrt(DK))
                        else:
                            for j in range(2):
                                c0 = c0s[j]
                                nc.scalar.activation(
                                    et[:, j * TQ + c0:(j + 1) * TQ],
                                    s_ps[:, j * TQ + c0:(j + 1) * TQ],
                                    mybir.ActivationFunctionType.Exp,
                                    scale=1.0 / np.sqrt(DK))
                        for j in range(2):
                            t = 2 * tp2 + j
                            if ds[j] is not None:
                                c0 = c0s[j]
                                # causal diag block: zero cols < row, on the
                                # otherwise-idle Pool engine (keeps DVE off
                                # the exp->mask->O critical chain)
                                nc.gpsimd.affine_select(
                                    out=et[:, j * TQ + c0:j * TQ + c0 + 128],
                                    in_=et[:, j * TQ + c0:j * TQ + c0 + 128],
                                    compare_op=mybir.AluOpType.is_ge,
                                    fill=0.0, base=0, channel_multiplier=-1,
                                    pattern=[[1, 128]])
                            elif mode == "masked":
                                mt = mtp.tile([128, TQ], BF16,
                                              name=f"mt{h}{s}{t}", tag="mt")
                                nc.sync.dma_start(
                                    out=mt,
                                    in_=maskd[t * 128:(t + 1) * 128,
                                              s * TQ:(s + 1) * TQ])
                                nc.vector.tensor_mul(
                                    et[:, j * TQ:(j + 1) * TQ],
                                    et[:, j * TQ:(j + 1) * TQ], mt[:])
                        if DBG and h == 0 and s == 0 and tp2 == 0:
                            nc.sync.dma_start(out=dbg["det"][:], in_=et[:])
                        etl[(h, s)].append(et)
                        padv((2 * TQ - c0s[0] - c0s[1]) * PE_CY)
                        drain(PAIR_BUDGET)

                def emit_O(h, s, opool, wide=False, norm_act=False):
                    """Flipped O-matmuls (et stationary, V_aug moving) +
                    per-q-tile normalization into stg."""
                    p, half = h // 2, h % 2
                    po = half * DK
                    force(("v", v_need(s)))
                    ets = etl.pop((h, s))
                    if wide:
                        o_ps = opool.tile([128, TQ], F32,
                                          name=f"o{h}_{s}", tag="op")
                    else:
                        o_ps = opool.tile([128, NQT * (DK + 1)], F32,
                                          name=f"o{h}_{s}", tag="o")
                    rcp = rcpp.tile([128, NQT], F32, name=f"r{h}_{s}",
                                    tag="rcp")
                    for qtl in range(NQT):
                        naq = _n_alive_qt(s, qtl, mode)
                        reg = o_ps[:, qtl * (DK + 1):(qtl + 1) * (DK + 1)]
                        for t in range(naq):
                            et = ets[t // 2]
                            j = t % 2
                            nc.tensor.matmul(
                                reg,
                                et[:, j * TQ + qtl * 128:
                                   j * TQ + (qtl + 1) * 128],
                                va[h][:, t * (DK + 1):(t + 1) * (DK + 1)],
                                start=(t == 0), stop=(t == naq - 1))
                        padv(naq * (DK + 1) * PE_CY)
                        drain(150.0)
                        if USE_DIV:
                            nc.vector.tensor_scalar(
                                out=stg[p][s][:, qtl * 128 + po:
                                              qtl * 128 + po + DK],
                                in0=reg[:, 0:DK],
                                scalar1=reg[:, DK:DK + 1],
                                scalar2=None,
                                op0=mybir.AluOpType.divide)
                    if not USE_DIV:
                        # one batched reciprocal over the 4 rowsums, then one
                        # per-partition-scalar multiply per q-tile (on ACT
                        # for the endgame slices, where the exp stream has
                        # drained and DVE is the congested engine)
                        nc.vector.reciprocal(
                            rcp[:], o_ps[:, DK:NQT * (DK + 1):DK + 1])
                        for qtl in range(NQT):
                            if norm_act:
                                nc.scalar.activation(
                                    stg[p][s][:, qtl * 128 + po:
                                              qtl * 128 + po + DK],
                                    o_ps[:, qtl * (DK + 1):
                                         qtl * (DK + 1) + DK],
                                    mybir.ActivationFunctionType.Copy,
                                    scale=rcp[:, qtl:qtl + 1])
                            else:
                                nc.vector.tensor_scalar_mul(
                                    stg[p][s][:, qtl * 128 + po:
                                              qtl * 128 + po + DK],
                                    o_ps[:, qtl * (DK + 1):
                                         qtl * (DK + 1) + DK],
                                    rcp[:, qtl:qtl + 1])
                    if DBG and h == 1 and s == 0:
                        nc.sync.dma_start(out=dbg["dstg00"][:],
                                          in_=stg[0][0][:])

                def emit_trans(s, m, fps, act=False):
                    """Pair transposes for q-tile m of slice s into an fps
                    slot (same byte size as the outproj psum -> shared tag),
                    then copies into otu (ACT for the final tail, where
                    the exp stream has drained and ACT idles)."""
                    tpt = fps.tile([128, D], BF16, name=f"tp{s}_{m}",
                                   tag="op")
                    for p in range(2):
                        nc.tensor.transpose(
                            tpt[:, p * 128:(p + 1) * 128],
                            stg[p][s][:, m * 128:(m + 1) * 128],
                            ident[:])
                    for p in range(2):
                        eng = nc.scalar if (act and p == 1) else nc.vector
                        if eng is nc.scalar:
                            nc.scalar.copy(
                                otu[p][s][:, m * 128:(m + 1) * 128],
                                tpt[:, p * 128:(p + 1) * 128])
                        else:
                            nc.vector.tensor_copy(
                                otu[p][s][:, m * 128:(m + 1) * 128],
                                tpt[:, p * 128:(p + 1) * 128])
                    padv(107.0)

                def emit_op(s, m, fps, act=False):
                    """Output projection for q-tile m of slice s. The final
                    tail's staging copies go to the otherwise-idle ACT."""
                    r0 = (s * NQT + m) * 128
                    o_sb = ostp.tile([128, D], BF16, name=f"os{s}_{m}",
                                     tag="os")
                    for n in range(2):
                        o_ps2 = fps.tile([128, TQ], F32,
                                         name=f"op{s}_{m}_{n}", tag="op")
                        for p in range(2):
                            nc.tensor.matmul(
                                o_ps2[:],
                                otu[p][s][:, m * 128:(m + 1) * 128],
                                wo_s[:, p * D + n * TQ:
                                     p * D + (n + 1) * TQ],
                                start=(p == 0), stop=(p == 1))
                        if act and n == 1:
                            nc.scalar.copy(
                                o_sb[:, n * TQ:(n + 1) * TQ], o_ps2[:])
                        else:
                            nc.vector.tensor_copy(
                                o_sb[:, n * TQ:(n + 1) * TQ], o_ps2[:])
                        nc.sync.dma_start(
                            out=out[r0:r0 + 128, n * TQ:(n + 1) * TQ],
                            in_=o_sb[:, n * TQ:(n + 1) * TQ])
                    padv(854.0)

                def enqueue_tail(s, fps, act=False):
                    """Transpose/outproj ladder enqueued as paced fillers
                    (drained between later S-pairs); transposes run one
                    q-tile ahead of the outproj so the otu copies are off
                    the PE wait chain."""
                    q_push(lambda: emit_trans(s, 0, fps, act), 107.0, 0.0)
                    for m in range(NQT):
                        if m + 1 < NQT:
                            q_push(lambda m=m: emit_trans(s, m + 1, fps,
                                                          act),
                                   107.0, 0.0)
                        q_push(lambda m=m: emit_op(s, m, fps, act),
                               854.0, 0.0)

                # ---- DMA emission + arrival estimates (DMA engines are a
                # single serial resource; the Tile scheduler preserves
                # per-engine emission order, so all overlap is hand-paced
                # with a filler queue driven by these estimates).
                DMA_LAT, SEM_LAT, FULL_T, HALF_T = 1850.0, 900.0, 1570.0, 800.0
                dma_t = [0.0]

                def dma_in(dst, src, ns):
                    nc.sync.dma_start(out=dst, in_=src)
                    dma_t[0] += ns
                    return DMA_LAT + dma_t[0] + SEM_LAT

                arr_wq = dma_in(wq_s[:], wq[:], FULL_T)
                arr_wk = dma_in(wk_s[:], wk[:], FULL_T)
                # q/k/v loaded in column-half waves interleaved so useful
                # work lands as early as possible: S(·,0/1) + all n<=1
                # projection groups need only columns 0:1024 of q/k, and
                # V tiles t<8 need only columns 0:1024 of v.
                qTr = [xbig.tile([128, T], IN_DT, name=f"qTr{kk}",
                                 tag=f"xq{kk}") for kk in range(NKT)]
                kTr = [xbig.tile([128, T], IN_DT, name=f"kTr{kk}",
                                 tag=f"xk{kk}") for kk in range(NKT)]
                vTr = [xbig.tile([128, T], IN_DT, name=f"vTr{kk}",
                                 tag=f"xv{kk}") for kk in range(NKT)]

                def half_wave(tiles, src, w):
                    lo, hi = w * (T // 2), (w + 1) * (T // 2)
                    return [dma_in(tiles[kk][:, lo:hi],
                                   src[kk * 128:(kk + 1) * 128, lo:hi],
                                   HALF_T) for kk in range(NKT)]

                arr_qh, arr_kh, arr_vw = [], [], []
                arr_qh.append(half_wave(qTr, qT, 0))
                arr_kh.append(half_wave(kTr, kTt, 0))
                arr_wv = dma_in(wv_s[:], wv[:], FULL_T)
                arr_vw.append(half_wave(vTr, vT, 0)[-1])
                arr_qh.append(half_wave(qTr, qT, 1))
                arr_kh.append(half_wave(kTr, kTt, 1))
                arr_vw.append(half_wave(vTr, vT, 1)[-1])
                dma_in(wo_s[:], wo[:], FULL_T)

                # ---- filler queue: (emit_fn, pe_ns, ready_ns, marker)
                import collections as _c
                queue = _c.deque()
                done = set()
                est_pe = [1500.0]
                warm_used = [0]
                wps = sps.tile([128, 2 * TQ], F32, name="wm", tag="s")

                def padv(ns):
                    est_pe[0] += ns

                def warm_one():
                    nc.tensor.matmul(wps[:, 0:128], warm[:], warm[:],
                                     start=True, stop=True)
                    warm_used[0] += 1
                    padv(55.0)

                def q_push(fn, pe_ns, ready, marker=None):
                    queue.append((fn, pe_ns, ready, marker))

                def drain(budget):
                    while budget > 0 and queue:
                        fn, pe_ns, ready, mk = queue[0]
                        if ready > est_pe[0] + 150:
                            break
                        queue.popleft()
                        fn()
                        est_pe[0] = max(est_pe[0], ready) + pe_ns
                        budget -= pe_ns
                        if mk:
                            done.add(mk)

                def force(marker):
                    while marker not in done and queue:
                        fn, pe_ns, ready, mk = queue[0]
                        if ready > est_pe[0] + 150 and \
                                warm_used[0] < N_WARMUP:
                            warm_one()
                            continue
                        queue.popleft()
                        fn()
                        est_pe[0] = max(est_pe[0], ready) + pe_ns
                        if mk:
                            done.add(mk)

                def drain_all():
                    while queue:
                        fn, pe_ns, ready, mk = queue.popleft()
                        fn()
                        est_pe[0] = max(est_pe[0], ready) + pe_ns
                        if mk:
                            done.add(mk)

                PJ = [None, "qk"]  # current proj psum pool + tag

                def proj_fillers(xr, w_s, dst, m, nA, nB, readys, mk):
                    tiles = {}

                    def mkf(n, kk):
                        def f():
                            if kk == 0:
                                tiles[n] = PJ[0].tile(
                                    [128, TQ], F32, name=f"pj{mk}{n}",
                                    tag=PJ[1])
                            nc.tensor.matmul(
                                tiles[n][:],
                                w_s[:, kk * G + m * 128:
                                    kk * G + (m + 1) * 128],
                                xr[kk][:, n * TQ:(n + 1) * TQ],
                                start=(kk == 0), stop=(kk == NKT - 1))
                            if kk == NKT - 1:
                                if mk == "k0" and n == 0:
                                    # split so S(0,0)'s first pair (key
                                    # cols 0:256) starts off the first half
                                    h2 = TQ // 2
                                    nc.vector.tensor_copy(
                                        dst[m][:, 0:h2], tiles[n][:, 0:h2])
                                    nc.vector.tensor_copy(
                                        dst[m][:, h2:TQ], tiles[n][:, h2:TQ])
                                else:
                                    nc.vector.tensor_copy(
                                        dst[m][:, n * TQ:(n + 1) * TQ],
                                        tiles[n][:])
                        return f
                    for kk in range(NKT):
                        for n in dict.fromkeys((nA, nB)):
                            q_push(mkf(n, kk), 216.0, readys(n, kk),
                                   marker=(mk, n) if kk == NKT - 1 else None)

                def v_fillers(ts):
                    cur = {}

                    def mkf(t, kk):
                        def f():
                            if kk == 0:
                                cur[t] = qkps.tile([128, TQ], F32,
                                                   name=f"v{t}", tag="qk")
                            nc.tensor.matmul(
                                cur[t][:, 0:G],
                                vTr[kk][:, t * 128:(t + 1) * 128],
                                wv_s[:, kk * G:(kk + 1) * G],
                                start=(kk == 0), stop=(kk == NKT - 1))
                            if kk == NKT - 1:
                                for h in range(HG):
                                    nc.vector.tensor_copy(
                                        va[h][:, t * (DK + 1):
                                              t * (DK + 1) + DK],
                                        cur[t][:, h * DK:(h + 1) * DK])
                        return f
                    for t in ts:
                        rdy = max(arr_wv, arr_vw[t // (NT // 2)])
                        for kk in range(NKT):
                            q_push(mkf(t, kk), 110.0, rdy,
                                   marker=("v", t) if kk == NKT - 1 else None)

                qkps_cm = tc.tile_pool(name="qkps", bufs=2, space="PSUM")
                qkps = qkps_cm.__enter__()
                PJ[0] = qkps
                oap_cm = tc.tile_pool(name="oap", bufs=2, space="PSUM")
                oap = oap_cm.__enter__()
                rq = lambda n, kk: max(arr_wq, arr_qh[n // 2][kk])
                rk = lambda n, kk: max(arr_wk, arr_kh[n // 2][kk])
                # half-0-dependent groups first (m0 and m1 n<=1), then V
                # t<8, then the half-1 groups; m1 n=3,2 defer to phase B.
                proj_fillers(qTr, wq_s, qt, 0, 0, 1, rq, "q0")
                proj_fillers(kTr, wk_s, kt, 0, 0, 1, rk, "k0")
                proj_fillers(qTr, wq_s, qt, 1, 0, 1, rq, "q1")
                proj_fillers(kTr, wk_s, kt, 1, 0, 1, rk, "k1")
                v_fillers(range(NT // 2))
                proj_fillers(qTr, wq_s, qt, 0, 2, 3, rq, "q0")
                proj_fillers(kTr, wk_s, kt, 0, 2, 3, rk, "k0")
                v_fillers(range(NT // 2, NT))

                def s_need(s):
                    return s if mode == "causal" else NS - 1

                def v_need(s):
                    return _n_alive(s, mode) - 1

                # phase A: heads 0+1 interleaved, ascending slices (slice s
                # needs only the n<=s projection groups; the filler queue
                # supplies m0/m1 projections and the V projection, paced
                # against the DMA wave arrivals)
                for s in range(NS):
                    if s == NS - 1:
                        # m1 n=3,2 projections: filler for the big final
                        # slice of this phase (PE-idle while ACT drains
                        # both heads' exps); must complete before the qkps
                        # pool closes (drain_all below)
                        proj_fillers(qTr, wq_s, qt, 1, 3, 2, rq, "q1")
                        proj_fillers(kTr, wk_s, kt, 1, 3, 2, rk, "k1")
                    force(("k0", s_need(s)))
                    emit_S(0, s)
                    if s > 0:
                        emit_O(1, s - 1, oap)
                    emit_S(1, s)
                    emit_O(0, s, oap)
                emit_O(1, NS - 1, oap)
                drain_all()
                oap_cm.__exit__(None, None, None)
                qkps_cm.__exit__(None, None, None)
                # heads 2+3 merged, slices largest-first, with the
                # transpose/outproj tail enqueued as the PE filler: the
                # phase stays PE-bound (S+O+tail vs two heads' exps),
                # absorbing per-slice exp stalls, and the final serial
                # cascade is the smallest slice. O-psums, transposes and
                # outproj psums share one 4-slot rotating pool so WAR
                # waits sit 4 requests back and copy latencies hide.
                morder = [1, NS - 1, NS - 2, 0] if NS == 4 \
                    else list(range(NS))
                with tc.tile_pool(name="fps", bufs=4, space="PSUM") as fps:
                    PJ[0], PJ[1] = fps, "op"
                    last = len(morder) - 1
                    for i, s in enumerate(morder):
                        force(("k1", s if mode == "causal" else 2))
                        emit_S(2, s)
                        if i > 0:
                            emit_O(3, morder[i - 1], fps, wide=True)
                        emit_S(3, s)
                        emit_O(2, s, fps, wide=True, norm_act=(i == last))
                        if i > 0:
                            enqueue_tail(morder[i - 1], fps,
                                         act=(i == last))
                    # let queued tail pieces cover the last exp's latency
                    # before the final O-block
                    drain(1500.0)
                    emit_O(3, morder[-1], fps, wide=True, norm_act=True)
                    enqueue_tail(morder[-1], fps, act=True)
                    drain_all()

            if DBG:
                nc.sync.dma_start(out=dbg["dqt0"][:], in_=qt[0][:])
                nc.sync.dma_start(out=dbg["dkt0"][:], in_=kt[0][:])
                nc.sync.dma_start(out=dbg["dva0"][:], in_=va[0][:])
                nc.sync.dma_start(out=dbg["dotu00"][:], in_=otu[0][0][:])

            otrs_cm.__exit__(None, None, None)

    split_multi_waits(nc)
    return nc


def _detect_mode(mask):
    if mask.all():
        return "dense"
    if np.array_equal(mask, np.tril(np.ones((T, T), dtype=bool))):
        return "causal"
    return "masked"


def kernel(q, k, v, mask, Wq, Wk, Wv, Wo, _trace=False, _trace_kwargs=None):
    q, k, v = np.asarray(q), np.asarray(k), np.asarray(v)
    Wq, Wk, Wv, Wo = (np.asarray(Wq), np.asarray(Wk),
                      np.asarray(Wv), np.asarray(Wo))
    mask = np.asarray(mask)
    mode = _detect_mode(mask)
    nc = build_program(mode)

    in_maps = []
    for c in range(8):
        b, g = c // 4, c % 4
        bf = ml_dtypes.bfloat16

        def packw(w):  # [D, G] -> [128, NKT*G] SBUF layout
            return np.ascontiguousarray(
                w.reshape(NKT, 128, G).transpose(1, 0, 2).reshape(128, NKT * G)
                .astype(bf))

        wo_sl = Wo[g * G:(g + 1) * G, :]
        im = {
            "qT": np.ascontiguousarray(q[b].T.astype(bf)),
            "kT": np.ascontiguousarray(k[b].T.astype(bf)),
            "vT": np.ascontiguousarray(v[b].T.astype(bf)),
            "wq": packw(Wq[:, g * G:(g + 1) * G]),
            "wk": packw(Wk[:, g * G:(g + 1) * G]),
            "wv": packw(Wv[:, g * G:(g + 1) * G]),
            "wo": np.ascontiguousarray(
                wo_sl.reshape(2, 128, D).transpose(1, 0, 2).reshape(128, 2 * D)
                .astype(bf)),
        }
        if mode == "masked":
            im["maskT"] = np.ascontiguousarray(
                mask.T.astype(ml_dtypes.bfloat16))
        in_maps.append(im)

    res = run_bass_kernel_spmd(nc, in_maps, list(range(8)), trace=_trace,
                               **(_trace_kwargs or {}))
    outs = [np.asarray(res.results[c]["out"]).astype(np.float32)
            for c in range(8)]
    full = np.stack([outs[4 * b] + outs[4 * b + 1] + outs[4 * b + 2]
                     + outs[4 * b + 3] for b in range(B)])
    if _trace:
        return full, res
    return full
